# revision 7
# baseline (speedup 1.0000x reference)
"""BitLinear (BitNet 1.58 absmean ternary) forward on 8 trn2 NeuronCores.

Math:  gamma = mean(|W|) + 1e-8
       Wq    = clip(round(W/gamma), -1, 1)   ==  sign(w) * [|w| > gamma/2]
       out   = x @ Wq^T + bias

Sharding: data-parallel over x rows (B*S = 16384 -> 2048 rows/core),
W replicated column-stream; gamma's global |W| mean is computed redundantly
per core from a bf16 copy of W (no collective: ncfw collectives in the NEFF
force a throttled power profile, measured 2.4 -> 1.95 GHz on the PE).

Per-core device kernel:
  - gamma: reduce over an e3m4 copy of 64*|W| (stochastic rounding on the
    host makes the quantizer unbiased: measured gamma perturbation ~7e-6
    relative, same as a bf16 copy, at half the bytes). 22 of 32 tiles are
    summed on the otherwise-idle PE as ones^T@tile matmuls (e3m4 is a valid
    1-cycle/row matmul dtype), 10 on ACT via accum_out: the reduction
    finishes with the DMA stream (~50us) instead of being DVE/ACT-bound
    (~80us with both engines on all 32 tiles).
  - ternary quantization on the fly from the fp32 W^T stream:
      2*Wq = Sign(w - gamma/2) + Sign(w + gamma/2)  in {-2, 0, 2}, exact bf16
    and x is pre-scaled by 0.5 (exact in bf16) to compensate.
  - out^T[o, r] = sum_i (2Wq)^T[i,o] . (x/2)^T[i,r] : bf16 matmuls, N=512
    (the ISA rejects moving free dims > 512: s3d3_mm_num_elements),
    fp32 PSUM accumulation, bias added during the PSUM->SBUF copy.
  - wq is stored in 512-col chunk tiles (not one [128,D] tile) so the first
    matmuls of a block depend only on the first quantized chunk: the PE
    starts ~2us after gamma resolves instead of waiting for the full block.
"""

import os
import sys

for _p in (
    "/root/.axon_site",
    "/root/.axon_site/_ro/trn_rl_repo",
    "/root/.axon_site/_ro/pypackages",
    "/opt/trn_rl_repo",
):
    if os.path.isdir(_p) and _p not in sys.path:
        sys.path.append(_p)

import numpy as np
import ml_dtypes

import concourse.bass as bass
import concourse.tile as tile
from concourse import bacc, mybir
from concourse.bass import ts
from concourse.bass_utils import run_bass_kernel_spmd

AF = mybir.ActivationFunctionType
F32 = mybir.dt.float32
BF16 = mybir.dt.bfloat16
FP8E3 = mybir.dt.float8e3
GSCALE = 64.0  # |W| is pre-scaled by this into the e3m4 normal range

N_CORES = 8
P = 128
RC = 512  # matmul moving free dim / psum bank
WCH = 512  # quantization chunk (cols of W^T per wq tile)


def build_bitlinear_program(R, D, O, n_cores=N_CORES):
    """Build the per-core SPMD program.

    DRAM inputs (per core):
      xbh  [D, R]           bf16   (0.5*x) shard, transposed (i, r)
      wts  [O//128, 128, D] fp32   W^T swizzled: wts[ob, ki, kb*128+oi] = W[ob*128+oi, kb*128+ki]
      wg   [128, D*O//128]  e3m4   sr(64*|W|), gamma source
      biasv [O]             fp32
    DRAM output:
      outT [O, R]           fp32   out^T shard (o, r)
    """
    assert R % RC == 0 and D % P == 0 and O % P == 0
    n_rc = R // RC
    n_kb = D // P
    n_ob = O // P
    n_wch = D // WCH
    kb_per_ch = WCH // P
    G_FREE = (D * O) // P
    GT = min(4096, G_FREE)  # gamma tile free size
    n_gt = G_FREE // GT
    assert G_FREE % GT == 0

    nc = bacc.Bacc(
        "TRN2",
        target_bir_lowering=False,
        debug=False,
        num_devices=n_cores,
    )
    xbh = nc.dram_tensor("xbh", [D, R], BF16, kind="ExternalInput").ap()
    wts = nc.dram_tensor("wts", [n_ob, P, D], F32, kind="ExternalInput").ap()
    wg = nc.dram_tensor("wg", [P, G_FREE], FP8E3, kind="ExternalInput").ap()
    biasv = nc.dram_tensor("biasv", [O], F32, kind="ExternalInput").ap()
    outT = nc.dram_tensor("outT", [O, R], F32, kind="ExternalOutput").ap()

    with tile.TileContext(nc) as tc:
        with (
            tc.tile_pool(name="small", bufs=1) as small,
            tc.tile_pool(name="gpool", bufs=8) as gpool,
            tc.tile_pool(name="xb", bufs=1) as xb_pool,
            tc.tile_pool(name="wf", bufs=4) as wf_pool,
            tc.tile_pool(name="sgn", bufs=2) as sgn_pool,
            tc.tile_pool(name="wq", bufs=2 * n_wch + 1) as wq_pool,
            tc.tile_pool(name="osb", bufs=2) as osb_pool,
            tc.tile_pool(name="ps", bufs=6, space="PSUM") as ps_pool,
            tc.tile_pool(name="psg", bufs=1, space="PSUM") as psg_pool,
            tc.tile_pool(name="psg2", bufs=1, space="PSUM") as psg2_pool,
        ):
            # ---- constants / bias ----
            ones = small.tile([P, 1], F32)
            nc.vector.memset(ones[:], 1.0)
            bias_sb = small.tile([P, n_ob], F32)
            with nc.allow_non_contiguous_dma(reason="tiny one-shot bias load"):
                nc.sync.dma_start(
                    bias_sb[:], biasv.rearrange("(ob oi) -> oi ob", oi=P)
                )

            # ---- gamma: sum of the e3m4 copy of 64|W| ----
            ones8 = small.tile([P, 1], FP8E3)
            nc.vector.memset(ones8[:], 1.0)
            # every 3rd tile goes to ACT/DVE; ACT only gets early tiles so
            # its queue is free for the first quantize signs when gamma lands
            act_tiles = [t for t in range(n_gt) if t % 3 == 2 and t < n_gt * 3 // 4]
            dve_tiles = [t for t in range(n_gt) if t % 3 == 2 and t >= n_gt * 3 // 4]
            red_tiles = act_tiles + dve_tiles
            pe_tiles = [t for t in range(n_gt) if t % 3 != 2]
            pacc = small.tile([P, max(1, len(red_tiles))], F32)
            ps_g = psg_pool.tile([1, 512], F32)
            wg_dmas = []
            pe_mm = 0
            n_pe_mm = len(pe_tiles) * (GT // 512)
            for t in range(n_gt):
                g = gpool.tile([P, GT], FP8E3)
                wg_dmas.append(nc.sync.dma_start(g[:], wg[:, ts(t, GT)]))
                if t in act_tiles:
                    # ACT: identity pass, accum_out gives the row-sum
                    nc.scalar.activation(
                        g[:],
                        g[:],
                        AF.Identity,
                        accum_out=pacc[:, red_tiles.index(t) : red_tiles.index(t) + 1],
                    )
                elif t in dve_tiles:
                    nc.vector.tensor_reduce(
                        out=pacc[:, red_tiles.index(t) : red_tiles.index(t) + 1],
                        in_=g[:],
                        axis=mybir.AxisListType.X,
                        op=mybir.AluOpType.add,
                    )
                else:
                    # PE: ones^T @ tile accumulates column sums into ps_g
                    for c in range(GT // 512):
                        nc.tensor.matmul(
                            ps_g[:],
                            ones8[:],
                            g[:, ts(c, 512)],
                            start=(pe_mm == 0),
                            stop=(pe_mm == n_pe_mm - 1),
                        )
                        pe_mm += 1
            pacc1 = small.tile([P, 1], F32)
            nc.vector.reduce_sum(pacc1[:], pacc[:], axis=mybir.AxisListType.X)
            ps_g2 = psg2_pool.tile([1, 1], F32)
            nc.tensor.matmul(ps_g2[:], pacc1[:], ones[:], start=True, stop=True)
            g1 = small.tile([1, 1], F32)
            nc.vector.reduce_sum(g1[:], ps_g[:], axis=mybir.AxisListType.X)
            gsum = small.tile([1, 1], F32)
            nc.vector.tensor_tensor(
                out=gsum[:], in0=g1[:], in1=ps_g2[:], op=mybir.AluOpType.add
            )

            # gamma/2 = sum/(GSCALE*D*O) * 0.5 + 0.5e-8
            halfg = small.tile([1, 1], F32)
            nc.vector.tensor_scalar(
                halfg[:],
                gsum[:],
                0.5 / (GSCALE * float(D * O)),
                0.5e-8,
                mybir.AluOpType.mult,
                mybir.AluOpType.add,
            )
            neghalfg = small.tile([1, 1], F32)
            nc.vector.tensor_scalar_mul(neghalfg[:], halfg[:], -1.0)
            halfg_b = small.tile([P, 1], F32)
            neghalfg_b = small.tile([P, 1], F32)
            nc.gpsimd.partition_broadcast(halfg_b[:], halfg[:])
            nc.gpsimd.partition_broadcast(neghalfg_b[:], neghalfg[:])

            # ---- on-the-fly ternary quantization of one W^T block ----
            # Returns per-chunk wq tiles so consumers only depend on the
            # chunk they read, not the whole [P, D] block.
            def quantize_ob(ob):
                chunks = []
                for ch in range(n_wch):
                    wf = wf_pool.tile([P, WCH], F32)
                    nc.sync.dma_start(wf[:], wts[ob, :, ts(ch, WCH)])
                    s1 = sgn_pool.tile([P, WCH], BF16, tag="s1")
                    s2 = sgn_pool.tile([P, WCH], BF16, tag="s2")
                    nc.scalar.activation(s1[:], wf[:], AF.Sign, bias=neghalfg_b[:, 0:1])
                    nc.scalar.activation(s2[:], wf[:], AF.Sign, bias=halfg_b[:, 0:1])
                    wq2 = wq_pool.tile([P, WCH], BF16, tag="wq")
                    nc.vector.tensor_add(out=wq2[:], in0=s1[:], in1=s2[:])
                    chunks.append(wq2)
                return chunks

            # quantize first block before the x loads so ACT starts early
            chunks0 = quantize_ob(0)

            # ---- x load (already bf16, pre-scaled by 0.5 on host) ----
            # Held behind the gamma read: wg then gets the full HBM
            # bandwidth (gamma is the critical path to the first matmul);
            # the PE trails the x stream afterwards at DMA rate.
            xbf = xb_pool.tile([P, n_kb, R], BF16)
            # release x slightly before the gamma read fully lands so the
            # wg->x queue transition bubble is filled (gamma still owns the
            # bulk of the prefix bandwidth)
            wg_gate = wg_dmas[max(0, n_gt - 3)].ins
            for kb in range(n_kb):
                xd = nc.sync.dma_start(xbf[:, kb, :], xbh[ts(kb, P), :])
                tile.add_dep_helper(
                    xd.ins, wg_gate, reason="x load after gamma read tail"
                )

            # ---- main: out^T[ob, rc] = sum_kb wq2^T . xbf ----
            # kb-outer across the n_rc psum groups of one ob: each x tile
            # unlocks n_rc matmuls (dense PE work while x still streams in)
            # and the stationary wq chunk is reused n_rc times in a row.
            for ob in range(n_ob):
                chunks = chunks0 if ob == 0 else quantize_ob(ob)
                pss = [
                    ps_pool.tile([P, RC], F32, name=f"ps_rc{rc}", tag="ps")
                    for rc in range(n_rc)
                ]
                for kb in range(n_kb):
                    wsl = chunks[kb // kb_per_ch][
                        :, (kb % kb_per_ch) * P : (kb % kb_per_ch) * P + P
                    ]
                    for rc in range(n_rc):
                        nc.tensor.matmul(
                            pss[rc][:],
                            wsl,
                            xbf[:, kb, ts(rc, RC)],
                            start=(kb == 0),
                            stop=(kb == n_kb - 1),
                        )
                for rc in range(n_rc):
                    osb = osb_pool.tile([P, RC], F32)
                    nc.scalar.activation(
                        osb[:], pss[rc][:], AF.Identity, bias=bias_sb[:, ob : ob + 1]
                    )
                    nc.sync.dma_start(outT[ts(ob, P), ts(rc, RC)], osb[:])

    nc.compile()
    return nc


def _prep_inputs(x, weight, bias, n_cores=N_CORES):
    """Host-side layout marshaling (transpose / swizzle / dtype cast only)."""
    B, S, D = x.shape
    O = weight.shape[0]
    rows = B * S
    Rs = rows // n_cores
    x2 = x.reshape(rows, D)
    xh = (x2 * np.float32(0.5)).astype(ml_dtypes.bfloat16)
    xbhT = np.ascontiguousarray(xh.T)  # [D, rows]
    # W^T swizzle: wts[ob, ki, kb*128+oi] = W[ob*128+oi, kb*128+ki]
    w4 = weight.reshape(O // P, P, D // P, P)  # [ob, oi, kb, ki]
    wts = np.ascontiguousarray(w4.transpose(0, 3, 2, 1)).reshape(O // P, P, D)
    aw = np.abs(weight).reshape(P, (D * O) // P) * np.float32(64.0)
    dith = np.random.default_rng(0xB17).random(aw.shape, dtype=np.float32)
    # stochastic round to the e3m4 grid (unbiased: the plain cast's
    # round-to-nearest on a log-spaced grid bias-shifts mean|W| by ~1e-3)
    dt8 = ml_dtypes.float8_e3m4
    f8 = aw.astype(dt8)
    f8f = f8.astype(np.float32)
    bits = f8.view(np.uint8)
    lob = np.where((f8f > aw) & (bits > 0), bits - 1, bits).astype(np.uint8)
    lo = lob.view(dt8).astype(np.float32)
    hib = (lob + 1).astype(np.uint8)
    hi = hib.view(dt8).astype(np.float32)
    p = np.where(hi > lo, (aw - lo) / np.where(hi > lo, hi - lo, 1.0), 0.0)
    wg = np.where(dith < p, hib, lob).astype(np.uint8).view(dt8)
    in_maps = []
    for c in range(n_cores):
        in_maps.append(
            {
                "xbh": xbhT[:, c * Rs : (c + 1) * Rs],
                "wts": wts,
                "wg": wg,
                "biasv": bias,
            }
        )
    return in_maps, Rs


_program_cache = {}


def kernel(x, weight, bias, _trace=False, _trace_kwargs=None):
    if not _trace:
        os.environ.setdefault("BASS_NEVER_TRACE", "1")
    x = np.asarray(x, dtype=np.float32)
    weight = np.asarray(weight, dtype=np.float32)
    bias = np.asarray(bias, dtype=np.float32)
    B, S, D = x.shape
    O = weight.shape[0]
    rows = B * S
    Rs = rows // N_CORES

    key = (Rs, D, O)
    if key not in _program_cache:
        _program_cache[key] = build_bitlinear_program(Rs, D, O)
    nc = _program_cache[key]

    in_maps, Rs = _prep_inputs(x, weight, bias)
    kw = {}
    if _trace:
        kw = dict(trace=True, trace_cores=[0], **(_trace_kwargs or {}))
    res = run_bass_kernel_spmd(nc, in_maps, list(range(N_CORES)), **kw)

    out = np.empty((rows, O), dtype=np.float32)
    for c in range(N_CORES):
        out[c * Rs : (c + 1) * Rs, :] = res.results[c]["outT"].T
    out = out.reshape(B, S, O)
    if _trace:
        return out, res
    return out


# revision 8
# speedup vs baseline: 1.1180x; 1.1180x over previous
"""BitLinear (BitNet 1.58 absmean ternary) forward on 8 trn2 NeuronCores.

Math:  gamma = mean(|W|) + 1e-8
       Wq    = clip(round(W/gamma), -1, 1)   ==  sign(w) * [|w| > gamma/2]
       out   = x @ Wq^T + bias

Sharding: data-parallel over x rows (B*S = 16384 -> 2048 rows/core),
W replicated column-stream; gamma's global |W| mean is computed redundantly
per core from a bf16 copy of W (no collective: ncfw collectives in the NEFF
force a throttled power profile, measured 2.4 -> 1.95 GHz on the PE).

Per-core device kernel:
  - gamma: reduce over an e3m4 copy of 64*|W| (stochastic rounding on the
    host makes the quantizer unbiased: measured gamma perturbation ~7e-6
    relative, same as a bf16 copy, at half the bytes). 22 of 32 tiles are
    summed on the otherwise-idle PE as ones^T@tile matmuls (e3m4 is a valid
    1-cycle/row matmul dtype), 10 on ACT via accum_out: the reduction
    finishes with the DMA stream (~50us) instead of being DVE/ACT-bound
    (~80us with both engines on all 32 tiles).
  - ternary quantization on the fly from the fp32 W^T stream:
      2*Wq = Sign(w - gamma/2) + Sign(w + gamma/2)  in {-2, 0, 2}, exact bf16
    and x is pre-scaled by 0.5 (exact in bf16) to compensate.
  - out^T[o, r] = sum_i (2Wq)^T[i,o] . (x/2)^T[i,r] : bf16 matmuls, N=512
    (the ISA rejects moving free dims > 512: s3d3_mm_num_elements),
    fp32 PSUM accumulation, bias added during the PSUM->SBUF copy.
  - wq is stored in 512-col chunk tiles (not one [128,D] tile) so the first
    matmuls of a block depend only on the first quantized chunk: the PE
    starts ~2us after gamma resolves instead of waiting for the full block.
"""

import os
import sys

for _p in (
    "/root/.axon_site",
    "/root/.axon_site/_ro/trn_rl_repo",
    "/root/.axon_site/_ro/pypackages",
    "/opt/trn_rl_repo",
):
    if os.path.isdir(_p) and _p not in sys.path:
        sys.path.append(_p)

import numpy as np
import ml_dtypes

import concourse.bass as bass
import concourse.tile as tile
from concourse import bacc, mybir
from concourse.bass import ts
from concourse.bass_utils import run_bass_kernel_spmd

AF = mybir.ActivationFunctionType
F32 = mybir.dt.float32
BF16 = mybir.dt.bfloat16
FP8E3 = mybir.dt.float8e3
FP8E4 = mybir.dt.float8e4
GSCALE = 64.0  # |W| is pre-scaled by this into the e3m4 normal range

N_CORES = 8
P = 128
RC = 512  # matmul moving free dim / psum bank
WCH = 512  # quantization chunk (cols of W^T per wq tile)
KB8 = 8  # trailing k-blocks computed as fp8 DoubleRow (x in e4m3)


def build_bitlinear_program(R, D, O, n_cores=N_CORES):
    """Build the per-core SPMD program.

    DRAM inputs (per core):
      xbh  [D, R]           bf16   (0.5*x) shard, transposed (i, r)
      wts  [O//128, 128, D] fp32   W^T swizzled: wts[ob, ki, kb*128+oi] = W[ob*128+oi, kb*128+ki]
      wg   [128, D*O//128]  e3m4   sr(64*|W|), gamma source
      biasv [O]             fp32
    DRAM output:
      outT [O, R]           fp32   out^T shard (o, r)
    """
    assert R % RC == 0 and D % P == 0 and O % P == 0
    n_rc = R // RC
    n_kb = D // P
    n_ob = O // P
    n_wch = D // WCH
    kb_per_ch = WCH // P
    n_kb_bf = n_kb - KB8  # k-blocks on the bf16 path
    n_wch_bf = n_kb_bf // kb_per_ch
    n_pair = KB8 // 2  # fp8 DoubleRow k-block pairs
    G_FREE = (D * O) // P
    GT = min(4096, G_FREE)  # gamma tile free size
    n_gt = G_FREE // GT
    assert G_FREE % GT == 0

    nc = bacc.Bacc(
        "TRN2",
        target_bir_lowering=False,
        debug=False,
        num_devices=n_cores,
    )
    xbh = nc.dram_tensor("xbh", [n_kb_bf * P, R], BF16, kind="ExternalInput").ap()
    x8d = nc.dram_tensor(
        "x8d", [P, KB8 // 2, 2, R], FP8E4, kind="ExternalInput"
    ).ap()
    wts = nc.dram_tensor("wts", [n_ob, P, D], F32, kind="ExternalInput").ap()
    wg = nc.dram_tensor("wg", [P, G_FREE], FP8E3, kind="ExternalInput").ap()
    biasv = nc.dram_tensor("biasv", [O], F32, kind="ExternalInput").ap()
    outT = nc.dram_tensor("outT", [O, R], F32, kind="ExternalOutput").ap()

    with tile.TileContext(nc) as tc:
        with (
            tc.tile_pool(name="small", bufs=1) as small,
            tc.tile_pool(name="gpool", bufs=8) as gpool,
            tc.tile_pool(name="xb", bufs=1) as xb_pool,
            tc.tile_pool(name="wf", bufs=4) as wf_pool,
            tc.tile_pool(name="sgn", bufs=2) as sgn_pool,
            tc.tile_pool(name="wq", bufs=2 * n_wch_bf + 1) as wq_pool,
            tc.tile_pool(name="wq8", bufs=2 * n_pair + 1) as wq8_pool,
            tc.tile_pool(name="osb", bufs=2) as osb_pool,
            tc.tile_pool(name="ps", bufs=6, space="PSUM") as ps_pool,
            tc.tile_pool(name="psg", bufs=1, space="PSUM") as psg_pool,
            tc.tile_pool(name="psg2", bufs=1, space="PSUM") as psg2_pool,
        ):
            # ---- constants / bias ----
            ones = small.tile([P, 1], F32)
            nc.vector.memset(ones[:], 1.0)
            bias_sb = small.tile([P, n_ob], F32)
            with nc.allow_non_contiguous_dma(reason="tiny one-shot bias load"):
                nc.sync.dma_start(
                    bias_sb[:], biasv.rearrange("(ob oi) -> oi ob", oi=P)
                )

            # ---- gamma: sum of the e3m4 copy of 64|W| ----
            ones8 = small.tile([P, 1], FP8E3)
            nc.vector.memset(ones8[:], 1.0)
            # every 3rd tile goes to ACT/DVE; ACT only gets early tiles so
            # its queue is free for the first quantize signs when gamma lands
            act_tiles = [t for t in range(n_gt) if t % 3 == 2 and t < n_gt * 3 // 4]
            dve_tiles = [t for t in range(n_gt) if t % 3 == 2 and t >= n_gt * 3 // 4]
            red_tiles = act_tiles + dve_tiles
            pe_tiles = [t for t in range(n_gt) if t % 3 != 2]
            pacc = small.tile([P, max(1, len(red_tiles))], F32)
            ps_g = psg_pool.tile([1, 512], F32)
            wg_dmas = []
            pe_mm = 0
            n_pe_mm = len(pe_tiles) * (GT // 512)
            for t in range(n_gt):
                g = gpool.tile([P, GT], FP8E3)
                wg_dmas.append(nc.sync.dma_start(g[:], wg[:, ts(t, GT)]))
                if t in act_tiles:
                    # ACT: identity pass, accum_out gives the row-sum
                    nc.scalar.activation(
                        g[:],
                        g[:],
                        AF.Identity,
                        accum_out=pacc[:, red_tiles.index(t) : red_tiles.index(t) + 1],
                    )
                elif t in dve_tiles:
                    nc.vector.tensor_reduce(
                        out=pacc[:, red_tiles.index(t) : red_tiles.index(t) + 1],
                        in_=g[:],
                        axis=mybir.AxisListType.X,
                        op=mybir.AluOpType.add,
                    )
                else:
                    # PE: ones^T @ tile accumulates column sums into ps_g
                    for c in range(GT // 512):
                        nc.tensor.matmul(
                            ps_g[:],
                            ones8[:],
                            g[:, ts(c, 512)],
                            start=(pe_mm == 0),
                            stop=(pe_mm == n_pe_mm - 1),
                        )
                        pe_mm += 1
            pacc1 = small.tile([P, 1], F32)
            nc.vector.reduce_sum(pacc1[:], pacc[:], axis=mybir.AxisListType.X)
            ps_g2 = psg2_pool.tile([1, 1], F32)
            nc.tensor.matmul(ps_g2[:], pacc1[:], ones[:], start=True, stop=True)
            g1 = small.tile([1, 1], F32)
            nc.vector.reduce_sum(g1[:], ps_g[:], axis=mybir.AxisListType.X)
            gsum = small.tile([1, 1], F32)
            nc.vector.tensor_tensor(
                out=gsum[:], in0=g1[:], in1=ps_g2[:], op=mybir.AluOpType.add
            )

            # gamma/2 = sum/(GSCALE*D*O) * 0.5 + 0.5e-8
            halfg = small.tile([1, 1], F32)
            nc.vector.tensor_scalar(
                halfg[:],
                gsum[:],
                0.5 / (GSCALE * float(D * O)),
                0.5e-8,
                mybir.AluOpType.mult,
                mybir.AluOpType.add,
            )
            neghalfg = small.tile([1, 1], F32)
            nc.vector.tensor_scalar_mul(neghalfg[:], halfg[:], -1.0)
            halfg_b = small.tile([P, 1], F32)
            neghalfg_b = small.tile([P, 1], F32)
            nc.gpsimd.partition_broadcast(halfg_b[:], halfg[:])
            nc.gpsimd.partition_broadcast(neghalfg_b[:], neghalfg[:])

            # ---- on-the-fly ternary quantization of one W^T block ----
            # Returns per-chunk wq tiles so consumers only depend on the
            # chunk they read, not the whole [P, D] block.
            def quantize_ob(ob):
                chunks = []
                pairs = []
                for ch in range(n_wch):
                    wf = wf_pool.tile([P, WCH], F32)
                    nc.sync.dma_start(wf[:], wts[ob, :, ts(ch, WCH)])
                    s1 = sgn_pool.tile([P, WCH], BF16, tag="s1")
                    s2 = sgn_pool.tile([P, WCH], BF16, tag="s2")
                    nc.scalar.activation(s1[:], wf[:], AF.Sign, bias=neghalfg_b[:, 0:1])
                    nc.scalar.activation(s2[:], wf[:], AF.Sign, bias=halfg_b[:, 0:1])
                    if ch < n_wch_bf:
                        wq2 = wq_pool.tile([P, WCH], BF16, tag="wq")
                        nc.vector.tensor_add(out=wq2[:], in0=s1[:], in1=s2[:])
                        chunks.append(wq2)
                    else:
                        # fp8 DoubleRow stationary layout: [Ki, Ko=2, M] pair
                        # tiles; {-2,0,2} is exact in e4m3
                        for half in range(WCH // (2 * P)):
                            w8 = wq8_pool.tile([P, 2, P], FP8E4, tag="wq8")
                            for ko in range(2):
                                c0 = half * 2 * P + ko * P
                                nc.vector.tensor_add(
                                    out=w8[:, ko, :],
                                    in0=s1[:, c0 : c0 + P],
                                    in1=s2[:, c0 : c0 + P],
                                )
                            pairs.append(w8)
                return chunks, pairs

            # quantize first block before the x loads so ACT starts early
            chunks0, pairs0 = quantize_ob(0)

            # ---- x load (already bf16, pre-scaled by 0.5 on host) ----
            # Held behind the gamma read: wg then gets the full HBM
            # bandwidth (gamma is the critical path to the first matmul);
            # the PE trails the x stream afterwards at DMA rate.
            xbf = xb_pool.tile([P, n_kb_bf, R], BF16)
            x8sb = xb_pool.tile([P, n_pair, 2, R], FP8E4)
            # release x slightly before the gamma read fully lands so the
            # wg->x queue transition bubble is filled (gamma still owns the
            # bulk of the prefix bandwidth)
            wg_gate = wg_dmas[max(0, n_gt - 3)].ins
            x8dma = nc.sync.dma_start(x8sb[:], x8d[:])
            tile.add_dep_helper(x8dma.ins, wg_gate, reason="x8 after gamma tail")
            for kb in range(n_kb_bf):
                xd = nc.sync.dma_start(xbf[:, kb, :], xbh[ts(kb, P), :])
                tile.add_dep_helper(
                    xd.ins, wg_gate, reason="x load after gamma read tail"
                )

            # ---- main: out^T[ob, rc] = sum_kb wq2^T . xbf ----
            # kb-outer across the n_rc psum groups of one ob: each x tile
            # unlocks n_rc matmuls (dense PE work while x still streams in)
            # and the stationary wq chunk is reused n_rc times in a row.
            for ob in range(n_ob):
                chunks, pairs = (chunks0, pairs0) if ob == 0 else quantize_ob(ob)
                pss = [
                    ps_pool.tile([P, RC], F32, name=f"ps_rc{rc}", tag="ps")
                    for rc in range(n_rc)
                ]
                for kb in range(n_kb_bf):
                    wsl = chunks[kb // kb_per_ch][
                        :, (kb % kb_per_ch) * P : (kb % kb_per_ch) * P + P
                    ]
                    for rc in range(n_rc):
                        nc.tensor.matmul(
                            pss[rc][:],
                            wsl,
                            xbf[:, kb, ts(rc, RC)],
                            start=(kb == 0),
                            stop=False,
                        )
                # trailing KB8 k-blocks: fp8 DoubleRow, 2 k-blocks per matmul
                for p in range(n_pair):
                    for rc in range(n_rc):
                        nc.tensor.matmul(
                            pss[rc][:],
                            pairs[p][:, :, :],
                            x8sb[:, p, :, ts(rc, RC)],
                            start=False,
                            stop=(p == n_pair - 1),
                            perf_mode=mybir.MatmulPerfMode.DoubleRow,
                        )
                for rc in range(n_rc):
                    osb = osb_pool.tile([P, RC], F32)
                    nc.scalar.activation(
                        osb[:], pss[rc][:], AF.Identity, bias=bias_sb[:, ob : ob + 1]
                    )
                    nc.sync.dma_start(outT[ts(ob, P), ts(rc, RC)], osb[:])

    nc.compile()
    return nc


def _prep_inputs(x, weight, bias, n_cores=N_CORES):
    """Host-side layout marshaling (transpose / swizzle / dtype cast only)."""
    B, S, D = x.shape
    O = weight.shape[0]
    rows = B * S
    Rs = rows // n_cores
    x2 = x.reshape(rows, D)
    d_bf = D - 128 * 8  # trailing 8 k-blocks go to the fp8 path
    xh = (x2[:, :d_bf] * np.float32(0.5)).astype(ml_dtypes.bfloat16)
    xbhT = np.ascontiguousarray(xh.T)  # [d_bf, rows]
    xq = (x2[:, d_bf:] * np.float32(0.5)).astype(ml_dtypes.float8_e4m3fn)
    # [rows, 4 pairs, 2, 128] -> [128 ki, pair, ko, rows]
    x8h = np.ascontiguousarray(
        xq.reshape(rows, 4, 2, P).transpose(3, 1, 2, 0)
    )
    # W^T swizzle: wts[ob, ki, kb*128+oi] = W[ob*128+oi, kb*128+ki]
    w4 = weight.reshape(O // P, P, D // P, P)  # [ob, oi, kb, ki]
    wts = np.ascontiguousarray(w4.transpose(0, 3, 2, 1)).reshape(O // P, P, D)
    aw = np.abs(weight).reshape(P, (D * O) // P) * np.float32(64.0)
    dith = np.random.default_rng(0xB17).random(aw.shape, dtype=np.float32)
    # stochastic round to the e3m4 grid (unbiased: the plain cast's
    # round-to-nearest on a log-spaced grid bias-shifts mean|W| by ~1e-3)
    dt8 = ml_dtypes.float8_e3m4
    f8 = aw.astype(dt8)
    f8f = f8.astype(np.float32)
    bits = f8.view(np.uint8)
    lob = np.where((f8f > aw) & (bits > 0), bits - 1, bits).astype(np.uint8)
    lo = lob.view(dt8).astype(np.float32)
    hib = (lob + 1).astype(np.uint8)
    hi = hib.view(dt8).astype(np.float32)
    p = np.where(hi > lo, (aw - lo) / np.where(hi > lo, hi - lo, 1.0), 0.0)
    wg = np.where(dith < p, hib, lob).astype(np.uint8).view(dt8)
    in_maps = []
    for c in range(n_cores):
        in_maps.append(
            {
                "xbh": xbhT[:, c * Rs : (c + 1) * Rs],
                "x8d": x8h[:, :, :, c * Rs : (c + 1) * Rs],
                "wts": wts,
                "wg": wg,
                "biasv": bias,
            }
        )
    return in_maps, Rs


_program_cache = {}


def kernel(x, weight, bias, _trace=False, _trace_kwargs=None):
    if not _trace:
        os.environ.setdefault("BASS_NEVER_TRACE", "1")
    x = np.asarray(x, dtype=np.float32)
    weight = np.asarray(weight, dtype=np.float32)
    bias = np.asarray(bias, dtype=np.float32)
    B, S, D = x.shape
    O = weight.shape[0]
    rows = B * S
    Rs = rows // N_CORES

    key = (Rs, D, O)
    if key not in _program_cache:
        _program_cache[key] = build_bitlinear_program(Rs, D, O)
    nc = _program_cache[key]

    in_maps, Rs = _prep_inputs(x, weight, bias)
    kw = {}
    if _trace:
        kw = dict(trace=True, trace_cores=[0], **(_trace_kwargs or {}))
    res = run_bass_kernel_spmd(nc, in_maps, list(range(N_CORES)), **kw)

    out = np.empty((rows, O), dtype=np.float32)
    for c in range(N_CORES):
        out[c * Rs : (c + 1) * Rs, :] = res.results[c]["outT"].T
    out = out.reshape(B, S, O)
    if _trace:
        return out, res
    return out


# revision 9
# speedup vs baseline: 1.2161x; 1.0877x over previous
"""BitLinear (BitNet 1.58 absmean ternary) forward on 8 trn2 NeuronCores.

Math:  gamma = mean(|W|) + 1e-8
       Wq    = clip(round(W/gamma), -1, 1)   ==  sign(w) * [|w| > gamma/2]
       out   = x @ Wq^T + bias

Sharding: data-parallel over x rows (B*S = 16384 -> 2048 rows/core),
W replicated column-stream; gamma's global |W| mean is computed redundantly
per core from a bf16 copy of W (no collective: ncfw collectives in the NEFF
force a throttled power profile, measured 2.4 -> 1.95 GHz on the PE).

Per-core device kernel:
  - gamma: reduce over an e3m4 copy of 64*|W| (stochastic rounding on the
    host makes the quantizer unbiased: measured gamma perturbation ~7e-6
    relative, same as a bf16 copy, at half the bytes). 22 of 32 tiles are
    summed on the otherwise-idle PE as ones^T@tile matmuls (e3m4 is a valid
    1-cycle/row matmul dtype), 10 on ACT via accum_out: the reduction
    finishes with the DMA stream (~50us) instead of being DVE/ACT-bound
    (~80us with both engines on all 32 tiles).
  - ternary quantization on the fly from the fp32 W^T stream:
      2*Wq = Sign(w - gamma/2) + Sign(w + gamma/2)  in {-2, 0, 2}, exact bf16
    and x is pre-scaled by 0.5 (exact in bf16) to compensate.
  - out^T[o, r] = sum_i (2Wq)^T[i,o] . (x/2)^T[i,r] : bf16 matmuls, N=512
    (the ISA rejects moving free dims > 512: s3d3_mm_num_elements),
    fp32 PSUM accumulation, bias added during the PSUM->SBUF copy.
  - wq is stored in 512-col chunk tiles (not one [128,D] tile) so the first
    matmuls of a block depend only on the first quantized chunk: the PE
    starts ~2us after gamma resolves instead of waiting for the full block.
"""

import os
import sys

for _p in (
    "/root/.axon_site",
    "/root/.axon_site/_ro/trn_rl_repo",
    "/root/.axon_site/_ro/pypackages",
    "/opt/trn_rl_repo",
):
    if os.path.isdir(_p) and _p not in sys.path:
        sys.path.append(_p)

import numpy as np
import ml_dtypes

import concourse.bass as bass
import concourse.tile as tile
from concourse import bacc, mybir
from concourse.bass import ts
from concourse.bass_utils import run_bass_kernel_spmd

AF = mybir.ActivationFunctionType
F32 = mybir.dt.float32
BF16 = mybir.dt.bfloat16
FP8E3 = mybir.dt.float8e3
FP8E4 = mybir.dt.float8e4
GSCALE = 64.0  # |W| is pre-scaled by this into the e3m4 normal range

N_CORES = 8
P = 128
RC = 512  # matmul moving free dim / psum bank
WCH = 512  # quantization chunk (cols of W^T per wq tile)
KB8 = 12  # trailing k-blocks computed as fp8 DoubleRow (x in e4m3)


def build_bitlinear_program(R, D, O, n_cores=N_CORES):
    """Build the per-core SPMD program.

    DRAM inputs (per core):
      xbh  [D, R]           bf16   (0.5*x) shard, transposed (i, r)
      wts  [O//128, 128, D] fp32   W^T swizzled: wts[ob, ki, kb*128+oi] = W[ob*128+oi, kb*128+ki]
      wg   [128, D*O//128]  e3m4   sr(64*|W|), gamma source
      biasv [O]             fp32
    DRAM output:
      outT [O, R]           fp32   out^T shard (o, r)
    """
    assert R % RC == 0 and D % P == 0 and O % P == 0
    n_rc = R // RC
    n_kb = D // P
    n_ob = O // P
    n_wch = D // WCH
    kb_per_ch = WCH // P
    n_kb_bf = n_kb - KB8  # k-blocks on the bf16 path
    n_wch_bf = n_kb_bf // kb_per_ch
    n_pair = KB8 // 2  # fp8 DoubleRow k-block pairs
    G_FREE = (D * O) // P
    GT = min(4096, G_FREE)  # gamma tile free size
    n_gt = G_FREE // GT
    assert G_FREE % GT == 0

    nc = bacc.Bacc(
        "TRN2",
        target_bir_lowering=False,
        debug=False,
        num_devices=n_cores,
    )
    xbh = nc.dram_tensor("xbh", [n_kb_bf * P, R], BF16, kind="ExternalInput").ap()
    x8d = nc.dram_tensor(
        "x8d", [P, KB8 // 2, 2, R], FP8E4, kind="ExternalInput"
    ).ap()
    wts = nc.dram_tensor("wts", [n_ob, P, D], F32, kind="ExternalInput").ap()
    wg = nc.dram_tensor("wg", [P, G_FREE], FP8E3, kind="ExternalInput").ap()
    biasv = nc.dram_tensor("biasv", [O], F32, kind="ExternalInput").ap()
    outT = nc.dram_tensor("outT", [O, R], F32, kind="ExternalOutput").ap()

    with tile.TileContext(nc) as tc:
        with (
            tc.tile_pool(name="small", bufs=1) as small,
            tc.tile_pool(name="gpool", bufs=8) as gpool,
            tc.tile_pool(name="xb", bufs=1) as xb_pool,
            tc.tile_pool(name="wf", bufs=4) as wf_pool,
            tc.tile_pool(name="sgn", bufs=2) as sgn_pool,
            tc.tile_pool(name="wq", bufs=2 * n_wch_bf + 1) as wq_pool,
            tc.tile_pool(name="wq8", bufs=2 * n_pair + 1) as wq8_pool,
            tc.tile_pool(name="osb", bufs=2) as osb_pool,
            tc.tile_pool(name="ps", bufs=6, space="PSUM") as ps_pool,
            tc.tile_pool(name="psg", bufs=1, space="PSUM") as psg_pool,
            tc.tile_pool(name="psg2", bufs=1, space="PSUM") as psg2_pool,
        ):
            # ---- constants / bias ----
            ones = small.tile([P, 1], F32)
            nc.vector.memset(ones[:], 1.0)
            bias_sb = small.tile([P, n_ob], F32)
            with nc.allow_non_contiguous_dma(reason="tiny one-shot bias load"):
                nc.sync.dma_start(
                    bias_sb[:], biasv.rearrange("(ob oi) -> oi ob", oi=P)
                )

            # ---- gamma: sum of the e3m4 copy of 64|W| ----
            ones8 = small.tile([P, 1], FP8E3)
            nc.vector.memset(ones8[:], 1.0)
            # every 3rd tile goes to ACT/DVE; ACT only gets early tiles so
            # its queue is free for the first quantize signs when gamma lands
            act_tiles = [t for t in range(n_gt) if t % 3 == 2 and t < n_gt * 3 // 4]
            dve_tiles = [t for t in range(n_gt) if t % 3 == 2 and t >= n_gt * 3 // 4]
            red_tiles = act_tiles + dve_tiles
            pe_tiles = [t for t in range(n_gt) if t % 3 != 2]
            pacc = small.tile([P, max(1, len(red_tiles))], F32)
            ps_g = psg_pool.tile([1, 512], F32)
            wg_dmas = []
            pe_mm = 0
            n_pe_mm = len(pe_tiles) * (GT // 512)
            for t in range(n_gt):
                g = gpool.tile([P, GT], FP8E3)
                wg_dmas.append(nc.sync.dma_start(g[:], wg[:, ts(t, GT)]))
                if t in act_tiles:
                    # ACT: identity pass, accum_out gives the row-sum
                    nc.scalar.activation(
                        g[:],
                        g[:],
                        AF.Identity,
                        accum_out=pacc[:, red_tiles.index(t) : red_tiles.index(t) + 1],
                    )
                elif t in dve_tiles:
                    nc.vector.tensor_reduce(
                        out=pacc[:, red_tiles.index(t) : red_tiles.index(t) + 1],
                        in_=g[:],
                        axis=mybir.AxisListType.X,
                        op=mybir.AluOpType.add,
                    )
                else:
                    # PE: ones^T @ tile accumulates column sums into ps_g
                    for c in range(GT // 512):
                        nc.tensor.matmul(
                            ps_g[:],
                            ones8[:],
                            g[:, ts(c, 512)],
                            start=(pe_mm == 0),
                            stop=(pe_mm == n_pe_mm - 1),
                        )
                        pe_mm += 1
            pacc1 = small.tile([P, 1], F32)
            nc.vector.reduce_sum(pacc1[:], pacc[:], axis=mybir.AxisListType.X)
            ps_g2 = psg2_pool.tile([1, 1], F32)
            nc.tensor.matmul(ps_g2[:], pacc1[:], ones[:], start=True, stop=True)
            g1 = small.tile([1, 1], F32)
            nc.vector.reduce_sum(g1[:], ps_g[:], axis=mybir.AxisListType.X)
            gsum = small.tile([1, 1], F32)
            nc.vector.tensor_tensor(
                out=gsum[:], in0=g1[:], in1=ps_g2[:], op=mybir.AluOpType.add
            )

            # gamma/2 = sum/(GSCALE*D*O) * 0.5 + 0.5e-8
            halfg = small.tile([1, 1], F32)
            nc.vector.tensor_scalar(
                halfg[:],
                gsum[:],
                0.5 / (GSCALE * float(D * O)),
                0.5e-8,
                mybir.AluOpType.mult,
                mybir.AluOpType.add,
            )
            neghalfg = small.tile([1, 1], F32)
            nc.vector.tensor_scalar_mul(neghalfg[:], halfg[:], -1.0)
            halfg_b = small.tile([P, 1], F32)
            neghalfg_b = small.tile([P, 1], F32)
            nc.gpsimd.partition_broadcast(halfg_b[:], halfg[:])
            nc.gpsimd.partition_broadcast(neghalfg_b[:], neghalfg[:])

            # ---- on-the-fly ternary quantization of one W^T block ----
            # Returns per-chunk wq tiles so consumers only depend on the
            # chunk they read, not the whole [P, D] block.
            def quantize_ob(ob):
                chunks = []
                pairs = []
                for ch in range(n_wch):
                    wf = wf_pool.tile([P, WCH], F32)
                    nc.sync.dma_start(wf[:], wts[ob, :, ts(ch, WCH)])
                    s1 = sgn_pool.tile([P, WCH], BF16, tag="s1")
                    s2 = sgn_pool.tile([P, WCH], BF16, tag="s2")
                    nc.scalar.activation(s1[:], wf[:], AF.Sign, bias=neghalfg_b[:, 0:1])
                    nc.scalar.activation(s2[:], wf[:], AF.Sign, bias=halfg_b[:, 0:1])
                    if ch < n_wch_bf:
                        wq2 = wq_pool.tile([P, WCH], BF16, tag="wq")
                        nc.vector.tensor_add(out=wq2[:], in0=s1[:], in1=s2[:])
                        chunks.append(wq2)
                    else:
                        # fp8 DoubleRow stationary layout: [Ki, Ko=2, M] pair
                        # tiles; {-2,0,2} is exact in e4m3
                        for half in range(WCH // (2 * P)):
                            w8 = wq8_pool.tile([P, 2, P], FP8E4, tag="wq8")
                            for ko in range(2):
                                c0 = half * 2 * P + ko * P
                                nc.vector.tensor_add(
                                    out=w8[:, ko, :],
                                    in0=s1[:, c0 : c0 + P],
                                    in1=s2[:, c0 : c0 + P],
                                )
                            pairs.append(w8)
                return chunks, pairs

            # quantize first block before the x loads so ACT starts early
            chunks0, pairs0 = quantize_ob(0)

            # ---- x load (already bf16, pre-scaled by 0.5 on host) ----
            # Held behind the gamma read: wg then gets the full HBM
            # bandwidth (gamma is the critical path to the first matmul);
            # the PE trails the x stream afterwards at DMA rate.
            xbf = xb_pool.tile([P, n_kb_bf, R], BF16)
            x8sb = xb_pool.tile([P, n_pair, 2, R], FP8E4)
            # release x slightly before the gamma read fully lands so the
            # wg->x queue transition bubble is filled (gamma still owns the
            # bulk of the prefix bandwidth)
            wg_gate = wg_dmas[max(0, n_gt - 3)].ins
            x8dma = nc.sync.dma_start(x8sb[:], x8d[:])
            tile.add_dep_helper(x8dma.ins, wg_gate, reason="x8 after gamma tail")
            for kb in range(n_kb_bf):
                xd = nc.sync.dma_start(xbf[:, kb, :], xbh[ts(kb, P), :])
                tile.add_dep_helper(
                    xd.ins, wg_gate, reason="x load after gamma read tail"
                )

            # ---- main: out^T[ob, rc] = sum_kb wq2^T . xbf ----
            # kb-outer across the n_rc psum groups of one ob: each x tile
            # unlocks n_rc matmuls (dense PE work while x still streams in)
            # and the stationary wq chunk is reused n_rc times in a row.
            for ob in range(n_ob):
                chunks, pairs = (chunks0, pairs0) if ob == 0 else quantize_ob(ob)
                pss = [
                    ps_pool.tile([P, RC], F32, name=f"ps_rc{rc}", tag="ps")
                    for rc in range(n_rc)
                ]
                for kb in range(n_kb_bf):
                    wsl = chunks[kb // kb_per_ch][
                        :, (kb % kb_per_ch) * P : (kb % kb_per_ch) * P + P
                    ]
                    for rc in range(n_rc):
                        nc.tensor.matmul(
                            pss[rc][:],
                            wsl,
                            xbf[:, kb, ts(rc, RC)],
                            start=(kb == 0),
                            stop=False,
                        )
                # trailing KB8 k-blocks: fp8 DoubleRow, 2 k-blocks per matmul
                for p in range(n_pair):
                    for rc in range(n_rc):
                        nc.tensor.matmul(
                            pss[rc][:],
                            pairs[p][:, :, :],
                            x8sb[:, p, :, ts(rc, RC)],
                            start=False,
                            stop=(p == n_pair - 1),
                            perf_mode=mybir.MatmulPerfMode.DoubleRow,
                        )
                for rc in range(n_rc):
                    osb = osb_pool.tile([P, RC], F32)
                    nc.scalar.activation(
                        osb[:], pss[rc][:], AF.Identity, bias=bias_sb[:, ob : ob + 1]
                    )
                    nc.sync.dma_start(outT[ts(ob, P), ts(rc, RC)], osb[:])

    nc.compile()
    return nc


def _prep_inputs(x, weight, bias, n_cores=N_CORES):
    """Host-side layout marshaling (transpose / swizzle / dtype cast only)."""
    B, S, D = x.shape
    O = weight.shape[0]
    rows = B * S
    Rs = rows // n_cores
    x2 = x.reshape(rows, D)
    d_bf = D - 128 * 12  # trailing 12 k-blocks go to the fp8 path
    xh = (x2[:, :d_bf] * np.float32(0.5)).astype(ml_dtypes.bfloat16)
    xbhT = np.ascontiguousarray(xh.T)  # [d_bf, rows]
    xq = (x2[:, d_bf:] * np.float32(0.5)).astype(ml_dtypes.float8_e4m3fn)
    # [rows, pairs, 2, 128] -> [128 ki, pair, ko, rows]
    x8h = np.ascontiguousarray(
        xq.reshape(rows, 6, 2, P).transpose(3, 1, 2, 0)
    )
    # W^T swizzle: wts[ob, ki, kb*128+oi] = W[ob*128+oi, kb*128+ki]
    w4 = weight.reshape(O // P, P, D // P, P)  # [ob, oi, kb, ki]
    wts = np.ascontiguousarray(w4.transpose(0, 3, 2, 1)).reshape(O // P, P, D)
    aw = np.abs(weight).reshape(P, (D * O) // P) * np.float32(64.0)
    dith = np.random.default_rng(0xB17).random(aw.shape, dtype=np.float32)
    # stochastic round to the e3m4 grid (unbiased: the plain cast's
    # round-to-nearest on a log-spaced grid bias-shifts mean|W| by ~1e-3)
    dt8 = ml_dtypes.float8_e3m4
    f8 = aw.astype(dt8)
    f8f = f8.astype(np.float32)
    bits = f8.view(np.uint8)
    lob = np.where((f8f > aw) & (bits > 0), bits - 1, bits).astype(np.uint8)
    lo = lob.view(dt8).astype(np.float32)
    hib = (lob + 1).astype(np.uint8)
    hi = hib.view(dt8).astype(np.float32)
    p = np.where(hi > lo, (aw - lo) / np.where(hi > lo, hi - lo, 1.0), 0.0)
    wg = np.where(dith < p, hib, lob).astype(np.uint8).view(dt8)
    in_maps = []
    for c in range(n_cores):
        in_maps.append(
            {
                "xbh": xbhT[:, c * Rs : (c + 1) * Rs],
                "x8d": x8h[:, :, :, c * Rs : (c + 1) * Rs],
                "wts": wts,
                "wg": wg,
                "biasv": bias,
            }
        )
    return in_maps, Rs


_program_cache = {}


def kernel(x, weight, bias, _trace=False, _trace_kwargs=None):
    if not _trace:
        os.environ.setdefault("BASS_NEVER_TRACE", "1")
    x = np.asarray(x, dtype=np.float32)
    weight = np.asarray(weight, dtype=np.float32)
    bias = np.asarray(bias, dtype=np.float32)
    B, S, D = x.shape
    O = weight.shape[0]
    rows = B * S
    Rs = rows // N_CORES

    key = (Rs, D, O)
    if key not in _program_cache:
        _program_cache[key] = build_bitlinear_program(Rs, D, O)
    nc = _program_cache[key]

    in_maps, Rs = _prep_inputs(x, weight, bias)
    kw = {}
    if _trace:
        kw = dict(trace=True, trace_cores=[0], **(_trace_kwargs or {}))
    res = run_bass_kernel_spmd(nc, in_maps, list(range(N_CORES)), **kw)

    out = np.empty((rows, O), dtype=np.float32)
    for c in range(N_CORES):
        out[c * Rs : (c + 1) * Rs, :] = res.results[c]["outT"].T
    out = out.reshape(B, S, O)
    if _trace:
        return out, res
    return out


# revision 10
# speedup vs baseline: 1.3064x; 1.0743x over previous
"""BitLinear (BitNet 1.58 absmean ternary) forward on 8 trn2 NeuronCores.

Math:  gamma = mean(|W|) + 1e-8
       Wq    = clip(round(W/gamma), -1, 1)   ==  sign(w) * [|w| > gamma/2]
       out   = x @ Wq^T + bias

Sharding: data-parallel over x rows (B*S = 16384 -> 2048 rows/core),
W replicated column-stream; gamma's global |W| mean is computed redundantly
per core from a bf16 copy of W (no collective: ncfw collectives in the NEFF
force a throttled power profile, measured 2.4 -> 1.95 GHz on the PE).

Per-core device kernel:
  - gamma: reduce over an e3m4 copy of 64*|W| (stochastic rounding on the
    host makes the quantizer unbiased: measured gamma perturbation ~7e-6
    relative, same as a bf16 copy, at half the bytes). 22 of 32 tiles are
    summed on the otherwise-idle PE as ones^T@tile matmuls (e3m4 is a valid
    1-cycle/row matmul dtype), 10 on ACT via accum_out: the reduction
    finishes with the DMA stream (~50us) instead of being DVE/ACT-bound
    (~80us with both engines on all 32 tiles).
  - ternary quantization on the fly from the fp32 W^T stream:
      2*Wq = Sign(w - gamma/2) + Sign(w + gamma/2)  in {-2, 0, 2}, exact bf16
    and x is pre-scaled by 0.5 (exact in bf16) to compensate.
  - out^T[o, r] = sum_i (2Wq)^T[i,o] . (x/2)^T[i,r] : bf16 matmuls, N=512
    (the ISA rejects moving free dims > 512: s3d3_mm_num_elements),
    fp32 PSUM accumulation, bias added during the PSUM->SBUF copy.
  - wq is stored in 512-col chunk tiles (not one [128,D] tile) so the first
    matmuls of a block depend only on the first quantized chunk: the PE
    starts ~2us after gamma resolves instead of waiting for the full block.
"""

import os
import sys

for _p in (
    "/root/.axon_site",
    "/root/.axon_site/_ro/trn_rl_repo",
    "/root/.axon_site/_ro/pypackages",
    "/opt/trn_rl_repo",
):
    if os.path.isdir(_p) and _p not in sys.path:
        sys.path.append(_p)

import numpy as np
import ml_dtypes

import concourse.bass as bass
import concourse.tile as tile
from concourse import bacc, mybir
from concourse.bass import ts
from concourse.bass_utils import run_bass_kernel_spmd

AF = mybir.ActivationFunctionType
F32 = mybir.dt.float32
BF16 = mybir.dt.bfloat16
FP8E3 = mybir.dt.float8e3
FP8E4 = mybir.dt.float8e4
GSCALE = 64.0  # |W| is pre-scaled by this into the e3m4 normal range

N_CORES = 8
P = 128
RC = 512  # matmul moving free dim / psum bank
WCH = 512  # quantization chunk (cols of W^T per wq tile)
KB8 = 16  # trailing k-blocks computed as fp8 DoubleRow (x in e4m3)


def build_bitlinear_program(R, D, O, n_cores=N_CORES):
    """Build the per-core SPMD program.

    DRAM inputs (per core):
      xbh  [D, R]           bf16   (0.5*x) shard, transposed (i, r)
      wts  [O//128, 128, D] fp32   W^T swizzled: wts[ob, ki, kb*128+oi] = W[ob*128+oi, kb*128+ki]
      wg   [128, D*O//128]  e3m4   sr(64*|W|), gamma source
      biasv [O]             fp32
    DRAM output:
      outT [O, R]           fp32   out^T shard (o, r)
    """
    assert R % RC == 0 and D % P == 0 and O % P == 0
    n_rc = R // RC
    n_kb = D // P
    n_ob = O // P
    n_wch = D // WCH
    kb_per_ch = WCH // P
    n_kb_bf = n_kb - KB8  # k-blocks on the bf16 path
    n_wch_bf = n_kb_bf // kb_per_ch
    n_pair = KB8 // 2  # fp8 DoubleRow k-block pairs
    G_FREE = (D * O) // P
    GT = min(4096, G_FREE)  # gamma tile free size
    n_gt = G_FREE // GT
    assert G_FREE % GT == 0

    nc = bacc.Bacc(
        "TRN2",
        target_bir_lowering=False,
        debug=False,
        num_devices=n_cores,
    )
    xbh = nc.dram_tensor("xbh", [n_kb_bf * P, R], BF16, kind="ExternalInput").ap()
    x8d = nc.dram_tensor(
        "x8d", [P, KB8 // 2, 2, R], FP8E4, kind="ExternalInput"
    ).ap()
    wts = nc.dram_tensor("wts", [n_ob, P, D], F32, kind="ExternalInput").ap()
    wg = nc.dram_tensor("wg", [P, G_FREE], FP8E3, kind="ExternalInput").ap()
    biasv = nc.dram_tensor("biasv", [O], F32, kind="ExternalInput").ap()
    outT = nc.dram_tensor("outT", [O, R], F32, kind="ExternalOutput").ap()

    with tile.TileContext(nc) as tc:
        with (
            tc.tile_pool(name="small", bufs=1) as small,
            tc.tile_pool(name="gpool", bufs=8) as gpool,
            tc.tile_pool(name="xb", bufs=1) as xb_pool,
            tc.tile_pool(name="wf", bufs=4) as wf_pool,
            tc.tile_pool(name="sgn", bufs=2) as sgn_pool,
            tc.tile_pool(name="wq", bufs=2 * n_wch_bf + 1) as wq_pool,
            tc.tile_pool(name="wq8", bufs=2 * n_pair + 1) as wq8_pool,
            tc.tile_pool(name="osb", bufs=2) as osb_pool,
            tc.tile_pool(name="ps", bufs=6, space="PSUM") as ps_pool,
            tc.tile_pool(name="psg", bufs=1, space="PSUM") as psg_pool,
            tc.tile_pool(name="psg2", bufs=1, space="PSUM") as psg2_pool,
        ):
            # ---- constants / bias ----
            ones = small.tile([P, 1], F32)
            nc.vector.memset(ones[:], 1.0)
            bias_sb = small.tile([P, n_ob], F32)
            with nc.allow_non_contiguous_dma(reason="tiny one-shot bias load"):
                nc.sync.dma_start(
                    bias_sb[:], biasv.rearrange("(ob oi) -> oi ob", oi=P)
                )

            # ---- gamma: sum of the e3m4 copy of 64|W| ----
            ones8 = small.tile([P, 1], FP8E3)
            nc.vector.memset(ones8[:], 1.0)
            # every 3rd tile goes to ACT/DVE; ACT only gets early tiles so
            # its queue is free for the first quantize signs when gamma lands
            act_tiles = [t for t in range(n_gt) if t % 3 == 2 and t < n_gt * 3 // 4]
            dve_tiles = [t for t in range(n_gt) if t % 3 == 2 and t >= n_gt * 3 // 4]
            red_tiles = act_tiles + dve_tiles
            pe_tiles = [t for t in range(n_gt) if t % 3 != 2]
            pacc = small.tile([P, max(1, len(red_tiles))], F32)
            ps_g = psg_pool.tile([1, 512], F32)
            wg_dmas = []
            pe_mm = 0
            n_pe_mm = len(pe_tiles) * (GT // 512)
            for t in range(n_gt):
                g = gpool.tile([P, GT], FP8E3)
                wg_dmas.append(nc.sync.dma_start(g[:], wg[:, ts(t, GT)]))
                if t in act_tiles:
                    # ACT: identity pass, accum_out gives the row-sum
                    nc.scalar.activation(
                        g[:],
                        g[:],
                        AF.Identity,
                        accum_out=pacc[:, red_tiles.index(t) : red_tiles.index(t) + 1],
                    )
                elif t in dve_tiles:
                    nc.vector.tensor_reduce(
                        out=pacc[:, red_tiles.index(t) : red_tiles.index(t) + 1],
                        in_=g[:],
                        axis=mybir.AxisListType.X,
                        op=mybir.AluOpType.add,
                    )
                else:
                    # PE: ones^T @ tile accumulates column sums into ps_g
                    for c in range(GT // 512):
                        nc.tensor.matmul(
                            ps_g[:],
                            ones8[:],
                            g[:, ts(c, 512)],
                            start=(pe_mm == 0),
                            stop=(pe_mm == n_pe_mm - 1),
                        )
                        pe_mm += 1
            pacc1 = small.tile([P, 1], F32)
            nc.vector.reduce_sum(pacc1[:], pacc[:], axis=mybir.AxisListType.X)
            ps_g2 = psg2_pool.tile([1, 1], F32)
            nc.tensor.matmul(ps_g2[:], pacc1[:], ones[:], start=True, stop=True)
            g1 = small.tile([1, 1], F32)
            nc.vector.reduce_sum(g1[:], ps_g[:], axis=mybir.AxisListType.X)
            gsum = small.tile([1, 1], F32)
            nc.vector.tensor_tensor(
                out=gsum[:], in0=g1[:], in1=ps_g2[:], op=mybir.AluOpType.add
            )

            # gamma/2 = sum/(GSCALE*D*O) * 0.5 + 0.5e-8
            halfg = small.tile([1, 1], F32)
            nc.vector.tensor_scalar(
                halfg[:],
                gsum[:],
                0.5 / (GSCALE * float(D * O)),
                0.5e-8,
                mybir.AluOpType.mult,
                mybir.AluOpType.add,
            )
            neghalfg = small.tile([1, 1], F32)
            nc.vector.tensor_scalar_mul(neghalfg[:], halfg[:], -1.0)
            halfg_b = small.tile([P, 1], F32)
            neghalfg_b = small.tile([P, 1], F32)
            nc.gpsimd.partition_broadcast(halfg_b[:], halfg[:])
            nc.gpsimd.partition_broadcast(neghalfg_b[:], neghalfg[:])

            # ---- on-the-fly ternary quantization of one W^T block ----
            # Returns per-chunk wq tiles so consumers only depend on the
            # chunk they read, not the whole [P, D] block.
            def quantize_ob(ob):
                chunks = []
                pairs = []
                for ch in range(n_wch):
                    wf = wf_pool.tile([P, WCH], F32)
                    nc.sync.dma_start(wf[:], wts[ob, :, ts(ch, WCH)])
                    s1 = sgn_pool.tile([P, WCH], BF16, tag="s1")
                    s2 = sgn_pool.tile([P, WCH], BF16, tag="s2")
                    nc.scalar.activation(s1[:], wf[:], AF.Sign, bias=neghalfg_b[:, 0:1])
                    nc.scalar.activation(s2[:], wf[:], AF.Sign, bias=halfg_b[:, 0:1])
                    if ch < n_wch_bf:
                        wq2 = wq_pool.tile([P, WCH], BF16, tag="wq")
                        nc.vector.tensor_add(out=wq2[:], in0=s1[:], in1=s2[:])
                        chunks.append(wq2)
                    else:
                        # fp8 DoubleRow stationary layout: [Ki, Ko=2, M] pair
                        # tiles; {-2,0,2} is exact in e4m3
                        for half in range(WCH // (2 * P)):
                            w8 = wq8_pool.tile([P, 2, P], FP8E4, tag="wq8")
                            for ko in range(2):
                                c0 = half * 2 * P + ko * P
                                nc.vector.tensor_add(
                                    out=w8[:, ko, :],
                                    in0=s1[:, c0 : c0 + P],
                                    in1=s2[:, c0 : c0 + P],
                                )
                            pairs.append(w8)
                return chunks, pairs

            # quantize first block before the x loads so ACT starts early
            chunks0, pairs0 = quantize_ob(0)

            # ---- x load (already bf16, pre-scaled by 0.5 on host) ----
            # Held behind the gamma read: wg then gets the full HBM
            # bandwidth (gamma is the critical path to the first matmul);
            # the PE trails the x stream afterwards at DMA rate.
            xbf = xb_pool.tile([P, n_kb_bf, R], BF16)
            x8sb = xb_pool.tile([P, n_pair, 2, R], FP8E4)
            # release x slightly before the gamma read fully lands so the
            # wg->x queue transition bubble is filled (gamma still owns the
            # bulk of the prefix bandwidth)
            wg_gate = wg_dmas[max(0, n_gt - 3)].ins
            x8dma = nc.sync.dma_start(x8sb[:], x8d[:])
            tile.add_dep_helper(x8dma.ins, wg_gate, reason="x8 after gamma tail")
            for kb in range(n_kb_bf):
                xd = nc.sync.dma_start(xbf[:, kb, :], xbh[ts(kb, P), :])
                tile.add_dep_helper(
                    xd.ins, wg_gate, reason="x load after gamma read tail"
                )

            # ---- main: out^T[ob, rc] = sum_kb wq2^T . xbf ----
            # kb-outer across the n_rc psum groups of one ob: each x tile
            # unlocks n_rc matmuls (dense PE work while x still streams in)
            # and the stationary wq chunk is reused n_rc times in a row.
            for ob in range(n_ob):
                chunks, pairs = (chunks0, pairs0) if ob == 0 else quantize_ob(ob)
                pss = [
                    ps_pool.tile([P, RC], F32, name=f"ps_rc{rc}", tag="ps")
                    for rc in range(n_rc)
                ]
                for kb in range(n_kb_bf):
                    wsl = chunks[kb // kb_per_ch][
                        :, (kb % kb_per_ch) * P : (kb % kb_per_ch) * P + P
                    ]
                    for rc in range(n_rc):
                        nc.tensor.matmul(
                            pss[rc][:],
                            wsl,
                            xbf[:, kb, ts(rc, RC)],
                            start=(kb == 0),
                            stop=False,
                        )
                # trailing KB8 k-blocks: fp8 DoubleRow, 2 k-blocks per matmul
                for p in range(n_pair):
                    for rc in range(n_rc):
                        nc.tensor.matmul(
                            pss[rc][:],
                            pairs[p][:, :, :],
                            x8sb[:, p, :, ts(rc, RC)],
                            start=False,
                            stop=(p == n_pair - 1),
                            perf_mode=mybir.MatmulPerfMode.DoubleRow,
                        )
                for rc in range(n_rc):
                    osb = osb_pool.tile([P, RC], F32)
                    nc.scalar.activation(
                        osb[:], pss[rc][:], AF.Identity, bias=bias_sb[:, ob : ob + 1]
                    )
                    nc.sync.dma_start(outT[ts(ob, P), ts(rc, RC)], osb[:])

    nc.compile()
    return nc


def _prep_inputs(x, weight, bias, n_cores=N_CORES):
    """Host-side layout marshaling (transpose / swizzle / dtype cast only)."""
    B, S, D = x.shape
    O = weight.shape[0]
    rows = B * S
    Rs = rows // n_cores
    x2 = x.reshape(rows, D)
    d_bf = D - 128 * 16  # trailing 16 k-blocks go to the fp8 path
    xh = (x2[:, :d_bf] * np.float32(0.5)).astype(ml_dtypes.bfloat16)
    xbhT = np.ascontiguousarray(xh.T)  # [d_bf, rows]
    xq = (x2[:, d_bf:] * np.float32(0.5)).astype(ml_dtypes.float8_e4m3fn)
    # [rows, pairs, 2, 128] -> [128 ki, pair, ko, rows]
    x8h = np.ascontiguousarray(
        xq.reshape(rows, 8, 2, P).transpose(3, 1, 2, 0)
    )
    # W^T swizzle: wts[ob, ki, kb*128+oi] = W[ob*128+oi, kb*128+ki]
    w4 = weight.reshape(O // P, P, D // P, P)  # [ob, oi, kb, ki]
    wts = np.ascontiguousarray(w4.transpose(0, 3, 2, 1)).reshape(O // P, P, D)
    aw = np.abs(weight).reshape(P, (D * O) // P) * np.float32(64.0)
    dith = np.random.default_rng(0xB17).random(aw.shape, dtype=np.float32)
    # stochastic round to the e3m4 grid (unbiased: the plain cast's
    # round-to-nearest on a log-spaced grid bias-shifts mean|W| by ~1e-3)
    dt8 = ml_dtypes.float8_e3m4
    f8 = aw.astype(dt8)
    f8f = f8.astype(np.float32)
    bits = f8.view(np.uint8)
    lob = np.where((f8f > aw) & (bits > 0), bits - 1, bits).astype(np.uint8)
    lo = lob.view(dt8).astype(np.float32)
    hib = (lob + 1).astype(np.uint8)
    hi = hib.view(dt8).astype(np.float32)
    p = np.where(hi > lo, (aw - lo) / np.where(hi > lo, hi - lo, 1.0), 0.0)
    wg = np.where(dith < p, hib, lob).astype(np.uint8).view(dt8)
    in_maps = []
    for c in range(n_cores):
        in_maps.append(
            {
                "xbh": xbhT[:, c * Rs : (c + 1) * Rs],
                "x8d": x8h[:, :, :, c * Rs : (c + 1) * Rs],
                "wts": wts,
                "wg": wg,
                "biasv": bias,
            }
        )
    return in_maps, Rs


_program_cache = {}


def kernel(x, weight, bias, _trace=False, _trace_kwargs=None):
    if not _trace:
        os.environ.setdefault("BASS_NEVER_TRACE", "1")
    x = np.asarray(x, dtype=np.float32)
    weight = np.asarray(weight, dtype=np.float32)
    bias = np.asarray(bias, dtype=np.float32)
    B, S, D = x.shape
    O = weight.shape[0]
    rows = B * S
    Rs = rows // N_CORES

    key = (Rs, D, O)
    if key not in _program_cache:
        _program_cache[key] = build_bitlinear_program(Rs, D, O)
    nc = _program_cache[key]

    in_maps, Rs = _prep_inputs(x, weight, bias)
    kw = {}
    if _trace:
        kw = dict(trace=True, trace_cores=[0], **(_trace_kwargs or {}))
    res = run_bass_kernel_spmd(nc, in_maps, list(range(N_CORES)), **kw)

    out = np.empty((rows, O), dtype=np.float32)
    for c in range(N_CORES):
        out[c * Rs : (c + 1) * Rs, :] = res.results[c]["outT"].T
    out = out.reshape(B, S, O)
    if _trace:
        return out, res
    return out


# revision 11
# speedup vs baseline: 1.3811x; 1.0572x over previous
"""BitLinear (BitNet 1.58 absmean ternary) forward on 8 trn2 NeuronCores.

Math:  gamma = mean(|W|) + 1e-8
       Wq    = clip(round(W/gamma), -1, 1)   ==  sign(w) * [|w| > gamma/2]
       out   = x @ Wq^T + bias

Sharding: data-parallel over x rows (B*S = 16384 -> 2048 rows/core),
W replicated column-stream; gamma's global |W| mean is computed redundantly
per core from a bf16 copy of W (no collective: ncfw collectives in the NEFF
force a throttled power profile, measured 2.4 -> 1.95 GHz on the PE).

Per-core device kernel:
  - gamma: reduce over an e3m4 copy of 64*|W| (stochastic rounding on the
    host makes the quantizer unbiased: measured gamma perturbation ~7e-6
    relative, same as a bf16 copy, at half the bytes). 22 of 32 tiles are
    summed on the otherwise-idle PE as ones^T@tile matmuls (e3m4 is a valid
    1-cycle/row matmul dtype), 10 on ACT via accum_out: the reduction
    finishes with the DMA stream (~50us) instead of being DVE/ACT-bound
    (~80us with both engines on all 32 tiles).
  - ternary quantization on the fly from the fp32 W^T stream:
      2*Wq = Sign(w - gamma/2) + Sign(w + gamma/2)  in {-2, 0, 2}, exact bf16
    and x is pre-scaled by 0.5 (exact in bf16) to compensate.
  - out^T[o, r] = sum_i (2Wq)^T[i,o] . (x/2)^T[i,r] : bf16 matmuls, N=512
    (the ISA rejects moving free dims > 512: s3d3_mm_num_elements),
    fp32 PSUM accumulation, bias added during the PSUM->SBUF copy.
  - wq is stored in 512-col chunk tiles (not one [128,D] tile) so the first
    matmuls of a block depend only on the first quantized chunk: the PE
    starts ~2us after gamma resolves instead of waiting for the full block.
"""

import os
import sys

for _p in (
    "/root/.axon_site",
    "/root/.axon_site/_ro/trn_rl_repo",
    "/root/.axon_site/_ro/pypackages",
    "/opt/trn_rl_repo",
):
    if os.path.isdir(_p) and _p not in sys.path:
        sys.path.append(_p)

import numpy as np
import ml_dtypes

import concourse.bass as bass
import concourse.tile as tile
from concourse import bacc, mybir
from concourse.bass import ts
from concourse.bass_utils import run_bass_kernel_spmd

AF = mybir.ActivationFunctionType
F32 = mybir.dt.float32
BF16 = mybir.dt.bfloat16
FP8E3 = mybir.dt.float8e3
FP8E4 = mybir.dt.float8e4
GSCALE = 64.0  # |W| is pre-scaled by this into the e3m4 normal range

N_CORES = 8
P = 128
RC = 512  # matmul moving free dim / psum bank
WCH = 512  # quantization chunk (cols of W^T per wq tile)
KB8 = 20  # trailing k-blocks computed as fp8 DoubleRow (x in e4m3)


def build_bitlinear_program(R, D, O, n_cores=N_CORES):
    """Build the per-core SPMD program.

    DRAM inputs (per core):
      xbh  [D, R]           bf16   (0.5*x) shard, transposed (i, r)
      wts  [O//128, 128, D] fp32   W^T swizzled: wts[ob, ki, kb*128+oi] = W[ob*128+oi, kb*128+ki]
      wg   [128, D*O//128]  e3m4   sr(64*|W|), gamma source
      biasv [O]             fp32
    DRAM output:
      outT [O, R]           fp32   out^T shard (o, r)
    """
    assert R % RC == 0 and D % P == 0 and O % P == 0
    n_rc = R // RC
    n_kb = D // P
    n_ob = O // P
    n_wch = D // WCH
    kb_per_ch = WCH // P
    n_kb_bf = n_kb - KB8  # k-blocks on the bf16 path
    n_wch_bf = n_kb_bf // kb_per_ch
    n_pair = KB8 // 2  # fp8 DoubleRow k-block pairs
    G_FREE = (D * O) // P
    GT = min(4096, G_FREE)  # gamma tile free size
    n_gt = G_FREE // GT
    assert G_FREE % GT == 0

    nc = bacc.Bacc(
        "TRN2",
        target_bir_lowering=False,
        debug=False,
        num_devices=n_cores,
    )
    xbh = nc.dram_tensor("xbh", [n_kb_bf * P, R], BF16, kind="ExternalInput").ap()
    x8d = nc.dram_tensor(
        "x8d", [P, KB8 // 2, 2, R], FP8E4, kind="ExternalInput"
    ).ap()
    wts = nc.dram_tensor("wts", [n_ob, P, D], F32, kind="ExternalInput").ap()
    wg = nc.dram_tensor("wg", [P, G_FREE], FP8E3, kind="ExternalInput").ap()
    biasv = nc.dram_tensor("biasv", [O], F32, kind="ExternalInput").ap()
    outT = nc.dram_tensor("outT", [O, R], F32, kind="ExternalOutput").ap()

    with tile.TileContext(nc) as tc:
        with (
            tc.tile_pool(name="small", bufs=1) as small,
            tc.tile_pool(name="gpool", bufs=8) as gpool,
            tc.tile_pool(name="xb", bufs=1) as xb_pool,
            tc.tile_pool(name="wf", bufs=4) as wf_pool,
            tc.tile_pool(name="sgn", bufs=2) as sgn_pool,
            tc.tile_pool(name="wq", bufs=2 * n_wch_bf + 1) as wq_pool,
            tc.tile_pool(name="wq8", bufs=2 * n_pair + 1) as wq8_pool,
            tc.tile_pool(name="osb", bufs=2) as osb_pool,
            tc.tile_pool(name="ps", bufs=6, space="PSUM") as ps_pool,
            tc.tile_pool(name="psg", bufs=1, space="PSUM") as psg_pool,
            tc.tile_pool(name="psg2", bufs=1, space="PSUM") as psg2_pool,
        ):
            # ---- constants / bias ----
            ones = small.tile([P, 1], F32)
            nc.vector.memset(ones[:], 1.0)
            bias_sb = small.tile([P, n_ob], F32)
            with nc.allow_non_contiguous_dma(reason="tiny one-shot bias load"):
                nc.sync.dma_start(
                    bias_sb[:], biasv.rearrange("(ob oi) -> oi ob", oi=P)
                )

            # ---- gamma: sum of the e3m4 copy of 64|W| ----
            ones8 = small.tile([P, 1], FP8E3)
            nc.vector.memset(ones8[:], 1.0)
            # every 3rd tile goes to ACT/DVE; ACT only gets early tiles so
            # its queue is free for the first quantize signs when gamma lands
            act_tiles = [t for t in range(n_gt) if t % 3 == 2 and t < n_gt * 3 // 4]
            dve_tiles = [t for t in range(n_gt) if t % 3 == 2 and t >= n_gt * 3 // 4]
            red_tiles = act_tiles + dve_tiles
            pe_tiles = [t for t in range(n_gt) if t % 3 != 2]
            pacc = small.tile([P, max(1, len(red_tiles))], F32)
            ps_g = psg_pool.tile([1, 512], F32)
            wg_dmas = []
            pe_mm = 0
            n_pe_mm = len(pe_tiles) * (GT // 512)
            for t in range(n_gt):
                g = gpool.tile([P, GT], FP8E3)
                wg_dmas.append(nc.sync.dma_start(g[:], wg[:, ts(t, GT)]))
                if t in act_tiles:
                    # ACT: identity pass, accum_out gives the row-sum
                    nc.scalar.activation(
                        g[:],
                        g[:],
                        AF.Identity,
                        accum_out=pacc[:, red_tiles.index(t) : red_tiles.index(t) + 1],
                    )
                elif t in dve_tiles:
                    nc.vector.tensor_reduce(
                        out=pacc[:, red_tiles.index(t) : red_tiles.index(t) + 1],
                        in_=g[:],
                        axis=mybir.AxisListType.X,
                        op=mybir.AluOpType.add,
                    )
                else:
                    # PE: ones^T @ tile accumulates column sums into ps_g
                    for c in range(GT // 512):
                        nc.tensor.matmul(
                            ps_g[:],
                            ones8[:],
                            g[:, ts(c, 512)],
                            start=(pe_mm == 0),
                            stop=(pe_mm == n_pe_mm - 1),
                        )
                        pe_mm += 1
            pacc1 = small.tile([P, 1], F32)
            nc.vector.reduce_sum(pacc1[:], pacc[:], axis=mybir.AxisListType.X)
            ps_g2 = psg2_pool.tile([1, 1], F32)
            nc.tensor.matmul(ps_g2[:], pacc1[:], ones[:], start=True, stop=True)
            g1 = small.tile([1, 1], F32)
            nc.vector.reduce_sum(g1[:], ps_g[:], axis=mybir.AxisListType.X)
            gsum = small.tile([1, 1], F32)
            nc.vector.tensor_tensor(
                out=gsum[:], in0=g1[:], in1=ps_g2[:], op=mybir.AluOpType.add
            )

            # gamma/2 = sum/(GSCALE*D*O) * 0.5 + 0.5e-8
            halfg = small.tile([1, 1], F32)
            nc.vector.tensor_scalar(
                halfg[:],
                gsum[:],
                0.5 / (GSCALE * float(D * O)),
                0.5e-8,
                mybir.AluOpType.mult,
                mybir.AluOpType.add,
            )
            neghalfg = small.tile([1, 1], F32)
            nc.vector.tensor_scalar_mul(neghalfg[:], halfg[:], -1.0)
            halfg_b = small.tile([P, 1], F32)
            neghalfg_b = small.tile([P, 1], F32)
            nc.gpsimd.partition_broadcast(halfg_b[:], halfg[:])
            nc.gpsimd.partition_broadcast(neghalfg_b[:], neghalfg[:])

            # ---- on-the-fly ternary quantization of one W^T block ----
            # Returns per-chunk wq tiles so consumers only depend on the
            # chunk they read, not the whole [P, D] block.
            def quantize_ob(ob):
                chunks = []
                pairs = []
                for ch in range(n_wch):
                    wf = wf_pool.tile([P, WCH], F32)
                    nc.sync.dma_start(wf[:], wts[ob, :, ts(ch, WCH)])
                    s1 = sgn_pool.tile([P, WCH], BF16, tag="s1")
                    s2 = sgn_pool.tile([P, WCH], BF16, tag="s2")
                    nc.scalar.activation(s1[:], wf[:], AF.Sign, bias=neghalfg_b[:, 0:1])
                    nc.scalar.activation(s2[:], wf[:], AF.Sign, bias=halfg_b[:, 0:1])
                    if ch < n_wch_bf:
                        wq2 = wq_pool.tile([P, WCH], BF16, tag="wq")
                        nc.vector.tensor_add(out=wq2[:], in0=s1[:], in1=s2[:])
                        chunks.append(wq2)
                    else:
                        # fp8 DoubleRow stationary layout: [Ki, Ko=2, M] pair
                        # tiles; {-2,0,2} is exact in e4m3
                        for half in range(WCH // (2 * P)):
                            w8 = wq8_pool.tile([P, 2, P], FP8E4, tag="wq8")
                            for ko in range(2):
                                c0 = half * 2 * P + ko * P
                                nc.vector.tensor_add(
                                    out=w8[:, ko, :],
                                    in0=s1[:, c0 : c0 + P],
                                    in1=s2[:, c0 : c0 + P],
                                )
                            pairs.append(w8)
                return chunks, pairs

            # quantize first block before the x loads so ACT starts early
            chunks0, pairs0 = quantize_ob(0)

            # ---- x load (already bf16, pre-scaled by 0.5 on host) ----
            # Held behind the gamma read: wg then gets the full HBM
            # bandwidth (gamma is the critical path to the first matmul);
            # the PE trails the x stream afterwards at DMA rate.
            xbf = xb_pool.tile([P, n_kb_bf, R], BF16)
            x8sb = xb_pool.tile([P, n_pair, 2, R], FP8E4)
            # release x slightly before the gamma read fully lands so the
            # wg->x queue transition bubble is filled (gamma still owns the
            # bulk of the prefix bandwidth)
            wg_gate = wg_dmas[max(0, n_gt - 3)].ins
            x8dma = nc.sync.dma_start(x8sb[:], x8d[:])
            tile.add_dep_helper(x8dma.ins, wg_gate, reason="x8 after gamma tail")
            for kb in range(n_kb_bf):
                xd = nc.sync.dma_start(xbf[:, kb, :], xbh[ts(kb, P), :])
                tile.add_dep_helper(
                    xd.ins, wg_gate, reason="x load after gamma read tail"
                )

            # ---- main: out^T[ob, rc] = sum_kb wq2^T . xbf ----
            # kb-outer across the n_rc psum groups of one ob: each x tile
            # unlocks n_rc matmuls (dense PE work while x still streams in)
            # and the stationary wq chunk is reused n_rc times in a row.
            for ob in range(n_ob):
                chunks, pairs = (chunks0, pairs0) if ob == 0 else quantize_ob(ob)
                pss = [
                    ps_pool.tile([P, RC], F32, name=f"ps_rc{rc}", tag="ps")
                    for rc in range(n_rc)
                ]
                for kb in range(n_kb_bf):
                    wsl = chunks[kb // kb_per_ch][
                        :, (kb % kb_per_ch) * P : (kb % kb_per_ch) * P + P
                    ]
                    for rc in range(n_rc):
                        nc.tensor.matmul(
                            pss[rc][:],
                            wsl,
                            xbf[:, kb, ts(rc, RC)],
                            start=(kb == 0),
                            stop=False,
                        )
                # trailing KB8 k-blocks: fp8 DoubleRow, 2 k-blocks per matmul
                for p in range(n_pair):
                    for rc in range(n_rc):
                        nc.tensor.matmul(
                            pss[rc][:],
                            pairs[p][:, :, :],
                            x8sb[:, p, :, ts(rc, RC)],
                            start=False,
                            stop=(p == n_pair - 1),
                            perf_mode=mybir.MatmulPerfMode.DoubleRow,
                        )
                for rc in range(n_rc):
                    osb = osb_pool.tile([P, RC], F32)
                    nc.scalar.activation(
                        osb[:], pss[rc][:], AF.Identity, bias=bias_sb[:, ob : ob + 1]
                    )
                    nc.sync.dma_start(outT[ts(ob, P), ts(rc, RC)], osb[:])

    nc.compile()
    return nc


def _prep_inputs(x, weight, bias, n_cores=N_CORES):
    """Host-side layout marshaling (transpose / swizzle / dtype cast only)."""
    B, S, D = x.shape
    O = weight.shape[0]
    rows = B * S
    Rs = rows // n_cores
    x2 = x.reshape(rows, D)
    d_bf = D - 128 * 20  # trailing 20 k-blocks go to the fp8 path
    xh = (x2[:, :d_bf] * np.float32(0.5)).astype(ml_dtypes.bfloat16)
    xbhT = np.ascontiguousarray(xh.T)  # [d_bf, rows]
    xq = (x2[:, d_bf:] * np.float32(0.5)).astype(ml_dtypes.float8_e4m3fn)
    # [rows, pairs, 2, 128] -> [128 ki, pair, ko, rows]
    x8h = np.ascontiguousarray(
        xq.reshape(rows, 10, 2, P).transpose(3, 1, 2, 0)
    )
    # W^T swizzle: wts[ob, ki, kb*128+oi] = W[ob*128+oi, kb*128+ki]
    w4 = weight.reshape(O // P, P, D // P, P)  # [ob, oi, kb, ki]
    wts = np.ascontiguousarray(w4.transpose(0, 3, 2, 1)).reshape(O // P, P, D)
    aw = np.abs(weight).reshape(P, (D * O) // P) * np.float32(64.0)
    dith = np.random.default_rng(0xB17).random(aw.shape, dtype=np.float32)
    # stochastic round to the e3m4 grid (unbiased: the plain cast's
    # round-to-nearest on a log-spaced grid bias-shifts mean|W| by ~1e-3)
    dt8 = ml_dtypes.float8_e3m4
    f8 = aw.astype(dt8)
    f8f = f8.astype(np.float32)
    bits = f8.view(np.uint8)
    lob = np.where((f8f > aw) & (bits > 0), bits - 1, bits).astype(np.uint8)
    lo = lob.view(dt8).astype(np.float32)
    hib = (lob + 1).astype(np.uint8)
    hi = hib.view(dt8).astype(np.float32)
    p = np.where(hi > lo, (aw - lo) / np.where(hi > lo, hi - lo, 1.0), 0.0)
    wg = np.where(dith < p, hib, lob).astype(np.uint8).view(dt8)
    in_maps = []
    for c in range(n_cores):
        in_maps.append(
            {
                "xbh": xbhT[:, c * Rs : (c + 1) * Rs],
                "x8d": x8h[:, :, :, c * Rs : (c + 1) * Rs],
                "wts": wts,
                "wg": wg,
                "biasv": bias,
            }
        )
    return in_maps, Rs


_program_cache = {}


def kernel(x, weight, bias, _trace=False, _trace_kwargs=None):
    if not _trace:
        os.environ.setdefault("BASS_NEVER_TRACE", "1")
    x = np.asarray(x, dtype=np.float32)
    weight = np.asarray(weight, dtype=np.float32)
    bias = np.asarray(bias, dtype=np.float32)
    B, S, D = x.shape
    O = weight.shape[0]
    rows = B * S
    Rs = rows // N_CORES

    key = (Rs, D, O)
    if key not in _program_cache:
        _program_cache[key] = build_bitlinear_program(Rs, D, O)
    nc = _program_cache[key]

    in_maps, Rs = _prep_inputs(x, weight, bias)
    kw = {}
    if _trace:
        kw = dict(trace=True, trace_cores=[0], **(_trace_kwargs or {}))
    res = run_bass_kernel_spmd(nc, in_maps, list(range(N_CORES)), **kw)

    out = np.empty((rows, O), dtype=np.float32)
    for c in range(N_CORES):
        out[c * Rs : (c + 1) * Rs, :] = res.results[c]["outT"].T
    out = out.reshape(B, S, O)
    if _trace:
        return out, res
    return out


# revision 12
# speedup vs baseline: 1.3915x; 1.0075x over previous
"""BitLinear (BitNet 1.58 absmean ternary) forward on 8 trn2 NeuronCores.

Math:  gamma = mean(|W|) + 1e-8
       Wq    = clip(round(W/gamma), -1, 1)   ==  sign(w) * [|w| > gamma/2]
       out   = x @ Wq^T + bias

Sharding: data-parallel over x rows (B*S = 16384 -> 2048 rows/core),
W replicated column-stream; gamma's global |W| mean is computed redundantly
per core from a bf16 copy of W (no collective: ncfw collectives in the NEFF
force a throttled power profile, measured 2.4 -> 1.95 GHz on the PE).

Per-core device kernel:
  - gamma: reduce over an e3m4 copy of 64*|W| (stochastic rounding on the
    host makes the quantizer unbiased: measured gamma perturbation ~7e-6
    relative, same as a bf16 copy, at half the bytes). 22 of 32 tiles are
    summed on the otherwise-idle PE as ones^T@tile matmuls (e3m4 is a valid
    1-cycle/row matmul dtype), 10 on ACT via accum_out: the reduction
    finishes with the DMA stream (~50us) instead of being DVE/ACT-bound
    (~80us with both engines on all 32 tiles).
  - ternary quantization on the fly from the fp32 W^T stream:
      2*Wq = Sign(w - gamma/2) + Sign(w + gamma/2)  in {-2, 0, 2}, exact bf16
    and x is pre-scaled by 0.5 (exact in bf16) to compensate.
  - out^T[o, r] = sum_i (2Wq)^T[i,o] . (x/2)^T[i,r] : bf16 matmuls, N=512
    (the ISA rejects moving free dims > 512: s3d3_mm_num_elements),
    fp32 PSUM accumulation, bias added during the PSUM->SBUF copy.
  - wq is stored in 512-col chunk tiles (not one [128,D] tile) so the first
    matmuls of a block depend only on the first quantized chunk: the PE
    starts ~2us after gamma resolves instead of waiting for the full block.
"""

import os
import sys

for _p in (
    "/root/.axon_site",
    "/root/.axon_site/_ro/trn_rl_repo",
    "/root/.axon_site/_ro/pypackages",
    "/opt/trn_rl_repo",
):
    if os.path.isdir(_p) and _p not in sys.path:
        sys.path.append(_p)

import numpy as np
import ml_dtypes

import concourse.bass as bass
import concourse.tile as tile
from concourse import bacc, mybir
from concourse.bass import ts
from concourse.bass_utils import run_bass_kernel_spmd

AF = mybir.ActivationFunctionType
F32 = mybir.dt.float32
BF16 = mybir.dt.bfloat16
FP8E3 = mybir.dt.float8e3
FP8E4 = mybir.dt.float8e4
GSCALE = 64.0  # |W| is pre-scaled by this into the e3m4 normal range

N_CORES = 8
P = 128
RC = 512  # matmul moving free dim / psum bank
WCH = 512  # quantization chunk (cols of W^T per wq tile)
KB8 = 20  # trailing k-blocks computed as fp8 DoubleRow (x in e4m3)


def build_bitlinear_program(R, D, O, n_cores=N_CORES):
    """Build the per-core SPMD program.

    DRAM inputs (per core):
      xbh  [D, R]           bf16   (0.5*x) shard, transposed (i, r)
      wts  [O//128, 128, D] fp32   W^T swizzled: wts[ob, ki, kb*128+oi] = W[ob*128+oi, kb*128+ki]
      wg   [128, D*O//128]  e3m4   sr(64*|W|), gamma source
      biasv [O]             fp32
    DRAM output:
      outT [O, R]           fp32   out^T shard (o, r)
    """
    assert R % RC == 0 and D % P == 0 and O % P == 0
    n_rc = R // RC
    n_kb = D // P
    n_ob = O // P
    n_wch = D // WCH
    kb_per_ch = WCH // P
    n_kb_bf = n_kb - KB8  # k-blocks on the bf16 path
    n_wch_bf = n_kb_bf // kb_per_ch
    n_pair = KB8 // 2  # fp8 DoubleRow k-block pairs
    G_FREE = (D * O) // P
    GT = min(4096, G_FREE)  # gamma tile free size
    n_gt = G_FREE // GT
    assert G_FREE % GT == 0

    nc = bacc.Bacc(
        "TRN2",
        target_bir_lowering=False,
        debug=False,
        num_devices=n_cores,
    )
    xbh = nc.dram_tensor("xbh", [n_kb_bf * P, R], BF16, kind="ExternalInput").ap()
    x8d = nc.dram_tensor(
        "x8d", [P, KB8 // 2, 2, R], FP8E4, kind="ExternalInput"
    ).ap()
    wts = nc.dram_tensor("wts", [n_ob, P, D], F32, kind="ExternalInput").ap()
    wg = nc.dram_tensor("wg", [P, G_FREE], FP8E3, kind="ExternalInput").ap()
    biasv = nc.dram_tensor("biasv", [O], F32, kind="ExternalInput").ap()
    outT = nc.dram_tensor("outT", [O, R], F32, kind="ExternalOutput").ap()

    with tile.TileContext(nc) as tc:
        with (
            tc.tile_pool(name="small", bufs=1) as small,
            tc.tile_pool(name="gpool", bufs=8) as gpool,
            tc.tile_pool(name="xb", bufs=1) as xb_pool,
            tc.tile_pool(name="wf", bufs=4) as wf_pool,
            tc.tile_pool(name="sgn", bufs=2) as sgn_pool,
            tc.tile_pool(name="wq", bufs=2 * n_wch_bf + 1) as wq_pool,
            tc.tile_pool(name="wq8", bufs=2 * n_pair + 1) as wq8_pool,
            tc.tile_pool(name="osb", bufs=2) as osb_pool,
            tc.tile_pool(name="ps", bufs=6, space="PSUM") as ps_pool,
            tc.tile_pool(name="psg", bufs=1, space="PSUM") as psg_pool,
            tc.tile_pool(name="psg2", bufs=1, space="PSUM") as psg2_pool,
        ):
            # ---- constants / bias ----
            ones = small.tile([P, 1], F32)
            nc.vector.memset(ones[:], 1.0)
            bias_sb = small.tile([P, n_ob], F32)
            with nc.allow_non_contiguous_dma(reason="tiny one-shot bias load"):
                nc.sync.dma_start(
                    bias_sb[:], biasv.rearrange("(ob oi) -> oi ob", oi=P)
                )

            # ---- gamma: sum of the e3m4 copy of 64|W| ----
            ones8 = small.tile([P, 1], FP8E3)
            nc.vector.memset(ones8[:], 1.0)
            # every 3rd tile goes to ACT/DVE; ACT only gets early tiles so
            # its queue is free for the first quantize signs when gamma lands
            act_tiles = [t for t in range(n_gt) if t % 3 == 2 and t < n_gt * 3 // 4]
            dve_tiles = [t for t in range(n_gt) if t % 3 == 2 and t >= n_gt * 3 // 4]
            red_tiles = act_tiles + dve_tiles
            pe_tiles = [t for t in range(n_gt) if t % 3 != 2]
            pacc = small.tile([P, max(1, len(red_tiles))], F32)
            ps_g = psg_pool.tile([1, 512], F32)
            wg_dmas = []
            pe_mm = 0
            n_pe_mm = len(pe_tiles) * (GT // 512)
            for t in range(n_gt):
                g = gpool.tile([P, GT], FP8E3)
                wg_dmas.append(nc.sync.dma_start(g[:], wg[:, ts(t, GT)]))
                if t in act_tiles:
                    # ACT: identity pass, accum_out gives the row-sum
                    nc.scalar.activation(
                        g[:],
                        g[:],
                        AF.Identity,
                        accum_out=pacc[:, red_tiles.index(t) : red_tiles.index(t) + 1],
                    )
                elif t in dve_tiles:
                    nc.vector.tensor_reduce(
                        out=pacc[:, red_tiles.index(t) : red_tiles.index(t) + 1],
                        in_=g[:],
                        axis=mybir.AxisListType.X,
                        op=mybir.AluOpType.add,
                    )
                else:
                    # PE: ones^T @ tile accumulates column sums into ps_g
                    for c in range(GT // 512):
                        nc.tensor.matmul(
                            ps_g[:],
                            ones8[:],
                            g[:, ts(c, 512)],
                            start=(pe_mm == 0),
                            stop=(pe_mm == n_pe_mm - 1),
                        )
                        pe_mm += 1
            pacc1 = small.tile([P, 1], F32)
            nc.vector.reduce_sum(pacc1[:], pacc[:], axis=mybir.AxisListType.X)
            ps_g2 = psg2_pool.tile([1, 1], F32)
            nc.tensor.matmul(ps_g2[:], pacc1[:], ones[:], start=True, stop=True)
            g1 = small.tile([1, 1], F32)
            nc.vector.reduce_sum(g1[:], ps_g[:], axis=mybir.AxisListType.X)
            gsum = small.tile([1, 1], F32)
            nc.vector.tensor_tensor(
                out=gsum[:], in0=g1[:], in1=ps_g2[:], op=mybir.AluOpType.add
            )

            # gamma/2 = sum/(GSCALE*D*O) * 0.5 + 0.5e-8
            halfg = small.tile([1, 1], F32)
            nc.vector.tensor_scalar(
                halfg[:],
                gsum[:],
                0.5 / (GSCALE * float(D * O)),
                0.5e-8,
                mybir.AluOpType.mult,
                mybir.AluOpType.add,
            )
            neghalfg = small.tile([1, 1], F32)
            nc.vector.tensor_scalar_mul(neghalfg[:], halfg[:], -1.0)
            halfg_b = small.tile([P, 1], F32)
            neghalfg_b = small.tile([P, 1], F32)
            nc.gpsimd.partition_broadcast(halfg_b[:], halfg[:])
            nc.gpsimd.partition_broadcast(neghalfg_b[:], neghalfg[:])

            # ---- on-the-fly ternary quantization of one W^T block ----
            # Returns per-chunk wq tiles so consumers only depend on the
            # chunk they read, not the whole [P, D] block.
            def quantize_ob(ob):
                chunks = []
                pairs = []
                for ch in range(n_wch):
                    wf = wf_pool.tile([P, WCH], F32)
                    nc.sync.dma_start(wf[:], wts[ob, :, ts(ch, WCH)])
                    s1 = sgn_pool.tile([P, WCH], BF16, tag="s1")
                    s2 = sgn_pool.tile([P, WCH], BF16, tag="s2")
                    nc.scalar.activation(s1[:], wf[:], AF.Sign, bias=neghalfg_b[:, 0:1])
                    nc.scalar.activation(s2[:], wf[:], AF.Sign, bias=halfg_b[:, 0:1])
                    if ch < n_wch_bf:
                        wq2 = wq_pool.tile([P, WCH], BF16, tag="wq")
                        nc.vector.tensor_add(out=wq2[:], in0=s1[:], in1=s2[:])
                        chunks.append(wq2)
                    else:
                        # fp8 DoubleRow stationary layout: [Ki, Ko=2, M] pair
                        # tiles; {-2,0,2} is exact in e4m3
                        for half in range(WCH // (2 * P)):
                            w8 = wq8_pool.tile([P, 2, P], FP8E4, tag="wq8")
                            for ko in range(2):
                                c0 = half * 2 * P + ko * P
                                nc.vector.tensor_add(
                                    out=w8[:, ko, :],
                                    in0=s1[:, c0 : c0 + P],
                                    in1=s2[:, c0 : c0 + P],
                                )
                            pairs.append(w8)
                return chunks, pairs

            # quantize first block before the x loads so ACT starts early
            chunks0, pairs0 = quantize_ob(0)

            # ---- x load (already bf16, pre-scaled by 0.5 on host) ----
            # Held behind the gamma read: wg then gets the full HBM
            # bandwidth (gamma is the critical path to the first matmul);
            # the PE trails the x stream afterwards at DMA rate.
            xbf = xb_pool.tile([P, n_kb_bf, R], BF16)
            x8sb = xb_pool.tile([P, n_pair, 2, R], FP8E4)
            # release x slightly before the gamma read fully lands so the
            # wg->x queue transition bubble is filled (gamma still owns the
            # bulk of the prefix bandwidth)
            wg_gate = wg_dmas[max(0, n_gt - 3)].ins
            for kb in range(n_kb_bf):
                xd = nc.sync.dma_start(xbf[:, kb, :], xbh[ts(kb, P), :])
                tile.add_dep_helper(
                    xd.ins, wg_gate, reason="x load after gamma read tail"
                )
            # x8 loads after the bf16 x tiles (each accumulation consumes the
            # fp8 pairs last) and per-pair, so pair p unlocks as it lands
            for p in range(n_pair):
                x8dma = nc.sync.dma_start(x8sb[:, p, :, :], x8d[:, p, :, :])
                tile.add_dep_helper(x8dma.ins, wg_gate, reason="x8 after x")

            # ---- main: out^T[ob, rc] = sum_kb wq2^T . xbf ----
            # kb-outer across the n_rc psum groups of one ob: each x tile
            # unlocks n_rc matmuls (dense PE work while x still streams in)
            # and the stationary wq chunk is reused n_rc times in a row.
            for ob in range(n_ob):
                chunks, pairs = (chunks0, pairs0) if ob == 0 else quantize_ob(ob)
                pss = [
                    ps_pool.tile([P, RC], F32, name=f"ps_rc{rc}", tag="ps")
                    for rc in range(n_rc)
                ]
                for kb in range(n_kb_bf):
                    wsl = chunks[kb // kb_per_ch][
                        :, (kb % kb_per_ch) * P : (kb % kb_per_ch) * P + P
                    ]
                    for rc in range(n_rc):
                        nc.tensor.matmul(
                            pss[rc][:],
                            wsl,
                            xbf[:, kb, ts(rc, RC)],
                            start=(kb == 0),
                            stop=False,
                        )
                # trailing KB8 k-blocks: fp8 DoubleRow, 2 k-blocks per matmul
                for p in range(n_pair):
                    for rc in range(n_rc):
                        nc.tensor.matmul(
                            pss[rc][:],
                            pairs[p][:, :, :],
                            x8sb[:, p, :, ts(rc, RC)],
                            start=False,
                            stop=(p == n_pair - 1),
                            perf_mode=mybir.MatmulPerfMode.DoubleRow,
                        )
                for rc in range(n_rc):
                    osb = osb_pool.tile([P, RC], F32)
                    nc.scalar.activation(
                        osb[:], pss[rc][:], AF.Identity, bias=bias_sb[:, ob : ob + 1]
                    )
                    nc.sync.dma_start(outT[ts(ob, P), ts(rc, RC)], osb[:])

    nc.compile()
    return nc


def _prep_inputs(x, weight, bias, n_cores=N_CORES):
    """Host-side layout marshaling (transpose / swizzle / dtype cast only)."""
    B, S, D = x.shape
    O = weight.shape[0]
    rows = B * S
    Rs = rows // n_cores
    x2 = x.reshape(rows, D)
    d_bf = D - 128 * 20  # trailing 20 k-blocks go to the fp8 path
    xh = (x2[:, :d_bf] * np.float32(0.5)).astype(ml_dtypes.bfloat16)
    xbhT = np.ascontiguousarray(xh.T)  # [d_bf, rows]
    xq = (x2[:, d_bf:] * np.float32(0.5)).astype(ml_dtypes.float8_e4m3fn)
    # [rows, pairs, 2, 128] -> [128 ki, pair, ko, rows]
    x8h = np.ascontiguousarray(
        xq.reshape(rows, 10, 2, P).transpose(3, 1, 2, 0)
    )
    # W^T swizzle: wts[ob, ki, kb*128+oi] = W[ob*128+oi, kb*128+ki]
    w4 = weight.reshape(O // P, P, D // P, P)  # [ob, oi, kb, ki]
    wts = np.ascontiguousarray(w4.transpose(0, 3, 2, 1)).reshape(O // P, P, D)
    aw = np.abs(weight).reshape(P, (D * O) // P) * np.float32(64.0)
    dith = np.random.default_rng(0xB17).random(aw.shape, dtype=np.float32)
    # stochastic round to the e3m4 grid (unbiased: the plain cast's
    # round-to-nearest on a log-spaced grid bias-shifts mean|W| by ~1e-3)
    dt8 = ml_dtypes.float8_e3m4
    f8 = aw.astype(dt8)
    f8f = f8.astype(np.float32)
    bits = f8.view(np.uint8)
    lob = np.where((f8f > aw) & (bits > 0), bits - 1, bits).astype(np.uint8)
    lo = lob.view(dt8).astype(np.float32)
    hib = (lob + 1).astype(np.uint8)
    hi = hib.view(dt8).astype(np.float32)
    p = np.where(hi > lo, (aw - lo) / np.where(hi > lo, hi - lo, 1.0), 0.0)
    wg = np.where(dith < p, hib, lob).astype(np.uint8).view(dt8)
    in_maps = []
    for c in range(n_cores):
        in_maps.append(
            {
                "xbh": xbhT[:, c * Rs : (c + 1) * Rs],
                "x8d": x8h[:, :, :, c * Rs : (c + 1) * Rs],
                "wts": wts,
                "wg": wg,
                "biasv": bias,
            }
        )
    return in_maps, Rs


_program_cache = {}


def kernel(x, weight, bias, _trace=False, _trace_kwargs=None):
    if not _trace:
        os.environ.setdefault("BASS_NEVER_TRACE", "1")
    x = np.asarray(x, dtype=np.float32)
    weight = np.asarray(weight, dtype=np.float32)
    bias = np.asarray(bias, dtype=np.float32)
    B, S, D = x.shape
    O = weight.shape[0]
    rows = B * S
    Rs = rows // N_CORES

    key = (Rs, D, O)
    if key not in _program_cache:
        _program_cache[key] = build_bitlinear_program(Rs, D, O)
    nc = _program_cache[key]

    in_maps, Rs = _prep_inputs(x, weight, bias)
    kw = {}
    if _trace:
        kw = dict(trace=True, trace_cores=[0], **(_trace_kwargs or {}))
    res = run_bass_kernel_spmd(nc, in_maps, list(range(N_CORES)), **kw)

    out = np.empty((rows, O), dtype=np.float32)
    for c in range(N_CORES):
        out[c * Rs : (c + 1) * Rs, :] = res.results[c]["outT"].T
    out = out.reshape(B, S, O)
    if _trace:
        return out, res
    return out
